# revision 2
# baseline (speedup 1.0000x reference)
"""GAU (Gated Attention Unit) Trainium2 Bass kernel, 8-core sequence-parallel.

Reference computation (all fp32):
    hid  = silu(x @ W_hidden + b_hidden);  v, gate = split(hid, 2)
    qk   = silu(x @ W_qk + b_qk)
    q    = qk * gamma[0] + beta[0];  k = qk * gamma[1] + beta[1]
    attn = relu((q @ k.T) / sqrt(dim))^2
    out  = ((attn @ v) * gate) @ W_out + b_out
    return out * x

Sharding (v2): rows (N=8192) split across 8 cores, 1024 rows each. Each core
computes k / v / q / gate for its OWN rows only, then AllGathers k and v
across the 8 cores. The collectives run on TOPSP+SDMA (separate silicon), so
they overlap with PE compute: AG(k) hides under the v/gate matmuls, AG(v)
hides under the gate matmuls + sim generation. This cuts per-core PE work
~40% vs replicating the k/v compute on every core.

All matmuls run as float32r (TF32-like, full PE rate at free-dim >= 256).
x is transposed on the host (xT_own input) so the kernel needs no transposes.
"""

import numpy as np

import concourse.bass as bass
import concourse.mybir as mybir
import concourse.tile as tile
from concourse import bacc

N = 8192          # total rows
D = 1024          # model dim
QK = 200          # qk dim
H = 2048          # hidden (v/gate) dim
NC = 8            # cores
R = N // NC       # rows per core
DT = D // 128     # d-tiles
HT = H // 128     # h-tiles
IC = R // 512     # i-chunks per core (own j-groups of 512)

f32 = mybir.dt.float32
f32r = mybir.dt.float32r
bf16 = mybir.dt.bfloat16
ACT = mybir.ActivationFunctionType
ALU = mybir.AluOpType
RG = [list(range(NC))]


def _build_nc(reps=1, vbias=False, obias=False,
              do_p1=True, do_pA=True, do_pB=True, do_pC=True):
    nc = bacc.Bacc("TRN2", target_bir_lowering=False, debug=False,
                   num_devices=NC)

    xT_own = nc.dram_tensor("xT_own", [D, R], f32r, kind="ExternalInput").ap()
    x_own = nc.dram_tensor("x_own", [R, D], f32, kind="ExternalInput").ap()
    w_h = nc.dram_tensor("w_h", [D, 2 * H], f32r, kind="ExternalInput").ap()
    w_qk = nc.dram_tensor("w_qk", [D, QK], f32r, kind="ExternalInput").ap()
    w_out = nc.dram_tensor("w_out", [H, D], f32r, kind="ExternalInput").ap()
    # per-c scalars, padded 200 -> [2, 128]
    gq = nc.dram_tensor("gq", [2, 128], f32, kind="ExternalInput").ap()
    bq = nc.dram_tensor("bq", [2, 128], f32, kind="ExternalInput").ap()
    gk = nc.dram_tensor("gk", [2, 128], f32, kind="ExternalInput").ap()
    bk = nc.dram_tensor("bk", [2, 128], f32, kind="ExternalInput").ap()
    bqk = nc.dram_tensor("bqk", [2, 128], f32, kind="ExternalInput").ap()
    bg = nc.dram_tensor("bg", [HT, 128], f32, kind="ExternalInput").ap()
    if vbias:
        bv = nc.dram_tensor("bv", [H], f32, kind="ExternalInput").ap()
    if obias:
        bo = nc.dram_tensor("bo", [D], f32, kind="ExternalInput").ap()
    out = nc.dram_tensor("out", [R, D], f32, kind="ExternalOutput").ap()

    with tile.TileContext(nc) as tc:
        with (
            tc.tile_pool(name="pers", bufs=1) as pers,
            tc.tile_pool(name="dram", bufs=1, space="DRAM") as dpool,
        ):
            # persistent small tiles
            gq_t = pers.tile([128, 2], f32)
            bq_t = pers.tile([128, 2], f32)
            gk_t = pers.tile([128, 2], f32)
            bk_t = pers.tile([128, 2], f32)
            bqk_t = pers.tile([128, 2], f32)
            bg_t = pers.tile([128, HT], f32)
            nc.sync.dma_start(out=gq_t, in_=gq.rearrange("ct c -> c ct"))
            nc.sync.dma_start(out=bq_t, in_=bq.rearrange("ct c -> c ct"))
            nc.sync.dma_start(out=gk_t, in_=gk.rearrange("ct c -> c ct"))
            nc.sync.dma_start(out=bk_t, in_=bk.rearrange("ct c -> c ct"))
            nc.sync.dma_start(out=bqk_t, in_=bqk.rearrange("ct c -> c ct"))
            nc.sync.dma_start(out=bg_t, in_=bg.rearrange("ht c -> c ht"))
            if vbias:
                bv_t = pers.tile([128, H], f32)
                nc.sync.dma_start(
                    out=bv_t,
                    in_=bass.AP(tensor=bv.tensor, offset=bv.offset,
                                ap=[[0, 128]] + list(bv.ap)),
                )
            if obias:
                bo_t = pers.tile([128, D], f32)
                nc.sync.dma_start(
                    out=bo_t,
                    in_=bass.AP(tensor=bo.tensor, offset=bo.offset,
                                ap=[[0, 128]] + list(bo.ap)),
                )

            # DRAM scratch: own-row slices (AG inputs, Local) and the
            # gathered full tensors (AG outputs, Shared)
            kT_own = dpool.tile([2, 128, R], f32, tag="kT_own")
            v_own = dpool.tile([R, H], bf16, tag="v_own")
            kT_ag = dpool.tile([NC, 2, 128, R], f32, tag="kT_ag",
                               addr_space="Shared")
            v_ag = dpool.tile([N, H], bf16, tag="v_ag", addr_space="Shared")
            gT_d = dpool.tile([HT, 128, R], f32, tag="gT_d")
            # qT lives in SBUF for the whole kernel (1 MB)
            qT_s = pers.tile([128, 2, R], f32r, tag="qT_s", name="qT_s")

            xT_r = xT_own.rearrange("(dt p) (jg j) -> p dt jg j", p=128, j=512)
            wh_r = w_h.rearrange("(dt p) h -> p dt h", p=128)
            wqk_r = w_qk.rearrange("(dt p) c -> p dt c", p=128)
            wo_r = w_out.rearrange("(ht p) m -> p ht m", p=128)
            xo_r = x_own.rearrange("(ic it p) m -> p ic it m", p=128, it=4)

            for rep in range(reps):
                if rep:
                    # full barrier between timing reps so SBUF/PSUM region
                    # reuse across the rep boundary is strictly ordered
                    tc.strict_bb_all_engine_barrier()
                # ============ phase 1: own-row k, q, v, gate + AGs ============
                if not do_p1:
                    pass
                else:
                 with (
                    tc.tile_pool(name="whp", bufs=1) as whp,
                    tc.tile_pool(name="xgp", bufs=1) as xgp,
                    tc.tile_pool(name="st1", bufs=(2 if vbias else 3)) as st1,
                    tc.tile_pool(name="ps_qk", bufs=2, space="PSUM") as ps_qk,
                    tc.tile_pool(name="ps_v", bufs=2, space="PSUM") as ps_v,
                    tc.tile_pool(name="ps_g", bufs=2, space="PSUM") as ps_g,
                ):
                    wqk_t = whp.tile([128, DT, QK], f32r, tag="wqk")
                    nc.sync.dma_start(out=wqk_t, in_=wqk_r)
                    # both own j-groups of x stay in SBUF all of phase 1
                    xg = xgp.tile([128, DT, IC, 512], f32r, tag="xg")
                    for jg in range(IC):
                        for dh in range(2):
                            eng = nc.sync if (jg + dh) % 2 else nc.gpsimd
                            eng.dma_start(
                                out=xg[:, dh * 4:(dh + 1) * 4, jg, :],
                                in_=xT_r[:, dh * 4:(dh + 1) * 4, jg, :])

                    # ---- qk -> k (own rows) + q (own rows) ----
                    for jg in range(IC):
                        for ct in range(2):
                            cw = 128 if ct == 0 else QK - 128
                            pq = ps_qk.tile([128, 512], f32)
                            for dt in range(DT):
                                nc.tensor.matmul(
                                    pq[:cw],
                                    wqk_t[:, dt, ct * 128:ct * 128 + cw],
                                    xg[:, dt, jg, :],
                                    start=(dt == 0),
                                    stop=(dt == DT - 1),
                                )
                            sil = st1.tile([128, 512], f32, tag="sil")
                            nc.scalar.activation(
                                sil[:cw], pq[:cw], ACT.Silu,
                                bias=bqk_t[:cw, ct:ct + 1],
                            )
                            kt = st1.tile([128, 512], f32, tag="kt")
                            nc.vector.tensor_scalar(
                                out=kt[:cw], in0=sil[:cw],
                                scalar1=gk_t[:cw, ct:ct + 1],
                                scalar2=bk_t[:cw, ct:ct + 1],
                                op0=ALU.mult, op1=ALU.add,
                            )
                            nc.sync.dma_start(
                                out=kT_own[ct, 0:cw, jg * 512:(jg + 1) * 512],
                                in_=kt[:cw],
                            )
                            nc.vector.tensor_scalar(
                                out=qT_s[:cw, ct, jg * 512:(jg + 1) * 512],
                                in0=sil[:cw],
                                scalar1=gq_t[:cw, ct:ct + 1],
                                scalar2=bq_t[:cw, ct:ct + 1],
                                op0=ALU.mult, op1=ALU.add,
                            )
                    # k slice done -> gather all k while v/gate compute
                    nc.gpsimd.collective_compute(
                        "AllGather", ALU.bypass, replica_groups=RG,
                        ins=[kT_own.opt()], outs=[kT_ag.opt()],
                    )

                    wh_t = whp.tile([128, DT, 2 * H], f32r, tag="wh")
                    for dt in range(DT):
                        eng = nc.sync if dt % 2 else nc.vector
                        eng.dma_start(out=wh_t[:, dt, :], in_=wh_r[:, dt, :])

                    # ---- v (own rows, row-major for the AG) ----
                    for jg in range(IC):
                        for jt in range(4):
                            for hc in range(4):
                                pv = ps_v.tile([128, 512], f32)
                                for dt in range(DT):
                                    nc.tensor.matmul(
                                        pv,
                                        xg[:, dt, jg, jt * 128:(jt + 1) * 128],
                                        wh_t[:, dt, hc * 512:(hc + 1) * 512],
                                        start=(dt == 0),
                                        stop=(dt == DT - 1),
                                    )
                                vt = st1.tile([128, 512], bf16, tag="vt")
                                if vbias:
                                    tmp = st1.tile([128, 512], f32, tag="vtmp")
                                    nc.vector.tensor_add(
                                        tmp, pv, bv_t[:, hc * 512:(hc + 1) * 512])
                                    nc.scalar.activation(vt, tmp, ACT.Silu)
                                else:
                                    nc.scalar.activation(vt, pv, ACT.Silu)
                                veng = nc.sync if (jt + hc) % 2 else nc.vector
                                veng.dma_start(
                                    out=v_own[(jg * 4 + jt) * 128:
                                              (jg * 4 + jt + 1) * 128,
                                              hc * 512:(hc + 1) * 512],
                                    in_=vt,
                                )
                    # v slice done -> gather all v while gate computes
                    nc.gpsimd.collective_compute(
                        "AllGather", ALU.bypass, replica_groups=RG,
                        ins=[v_own.opt()], outs=[v_ag.opt()],
                    )

                    # ---- gateT (own rows) ----
                    for jg in range(IC):
                        for ht in range(HT):
                            pg = ps_g.tile([128, 512], f32)
                            for dt in range(DT):
                                nc.tensor.matmul(
                                    pg,
                                    wh_t[:, dt, H + ht * 128:H + (ht + 1) * 128],
                                    xg[:, dt, jg, :],
                                    start=(dt == 0),
                                    stop=(dt == DT - 1),
                                )
                            gt = st1.tile([128, 512], f32, tag="gt")
                            nc.scalar.activation(
                                gt, pg, ACT.Silu, bias=bg_t[:, ht:ht + 1])
                            nc.sync.dma_start(
                                out=gT_d[ht, :, jg * 512:(jg + 1) * 512],
                                in_=gt,
                            )

                # ================= phase 2: attention per i-chunk =================
                with (
                    tc.tile_pool(name="p2sb", bufs=1) as p2sb,
                    tc.tile_pool(name="kqp", bufs=2) as kqp,
                    tc.tile_pool(name="vst", bufs=5) as vst,
                    tc.tile_pool(name="gst", bufs=2) as gst,
                    tc.tile_pool(name="wop", bufs=2) as wop,
                    tc.tile_pool(name="xop", bufs=1) as xop,
                    tc.tile_pool(name="ost", bufs=1) as osp,
                    tc.tile_pool(name="p2ps", bufs=1, space="PSUM") as p2ps,
                ):
                    # PSUM layout: tag "sim" = 2 banks (A), tag "acc" = 6 banks
                    # (B out1T h-groups of 6/6/4, C out2 4 i-tiles) -> 8 total,
                    # static, so A/B/C of consecutive i-chunks overlap freely.
                    for ic in range(IC):
                        attn = p2sb.tile([128, N // 128, 512], bf16,
                                         tag="attn", name="attn")
                        gated = p2sb.tile([128, HT, 512], f32r,
                                          tag="gated", name="gated")

                        # ---- A: attn[j, i-chunk] = relu(k.T q)^2 ----
                        if do_pA:
                            q_sb = qT_s[:, :, ic * 512:(ic + 1) * 512]
                            for jg in range(N // 512):
                                kt_sb = kqp.tile([128, 2, 512], f32r,
                                                 tag="kt_sb", name="kt_sb")
                                keng = nc.sync if jg % 2 else nc.gpsimd
                                keng.dma_start(
                                    out=kt_sb,
                                    in_=kT_ag[jg // 2, :, :,
                                              (jg % 2) * 512:(jg % 2 + 1) * 512]
                                    .rearrange("ct c j -> c ct j"),
                                )
                                for j4 in range(4):
                                    jt = jg * 4 + j4
                                    pss = p2ps.tile([128, 512], f32, tag="sim",
                                                    bufs=2, name="pss")
                                    nc.tensor.matmul(
                                        pss, kt_sb[:, 0, j4 * 128:(j4 + 1) * 128],
                                        q_sb[:, 0, :], start=True, stop=False)
                                    nc.tensor.matmul(
                                        pss, kt_sb[0:QK - 128, 1,
                                                   j4 * 128:(j4 + 1) * 128],
                                        q_sb[0:QK - 128, 1, :],
                                        start=False, stop=True)
                                    rel = kqp.tile([128, 512], f32,
                                                   tag="rel", bufs=2,
                                                   name="rel")
                                    nc.scalar.activation(rel, pss, ACT.Relu)
                                    nc.vector.tensor_mul(
                                        attn[:, jt, :], rel, rel)

                            if not (do_pB and do_pC):
                                pa = kqp.tile([128, 512], f32, tag="pa",
                                              bufs=1, name="pa")
                                nc.vector.tensor_copy(pa, attn[:, 0, :])
                                nc.sync.dma_start(
                                    out=out.rearrange("(a p) m -> p a m", p=128)
                                    [:, 1 + ic, 0:512], in_=pa)

                        # ---- B: out1T[h, i-chunk] = v-lhsT @ attn; * gateT ----
                        if do_pB:
                            for h0, nht in ((0, 6), (6, 6), (12, 4)):
                                po = p2ps.tile([128, nht, 512], f32, tag="acc",
                                               name="po")
                                for jt in range(N // 128):
                                    vt = vst.tile([128, nht * 128], bf16,
                                                  tag="vt", name="vt")
                                    half = nht * 64
                                    for dh in range(2):
                                        eng = nc.sync if (jt + dh) % 2 else nc.gpsimd
                                        eng.dma_start(
                                            out=vt[:, dh * half:(dh + 1) * half],
                                            in_=v_ag[jt * 128:(jt + 1) * 128,
                                                     h0 * 128 + dh * half:
                                                     h0 * 128 + (dh + 1) * half],
                                        )
                                    for hh in range(nht):
                                        nc.tensor.matmul(
                                            po[:, hh, :],
                                            vt[:, hh * 128:(hh + 1) * 128],
                                            attn[:, jt, :],
                                            start=(jt == 0),
                                            stop=(jt == N // 128 - 1),
                                        )
                                for hh in range(nht):
                                    ht = h0 + hh
                                    gt = gst.tile([128, 512], f32, tag="gt",
                                                  name="gt")
                                    geng = nc.sync if ht % 2 else nc.gpsimd
                                    geng.dma_start(
                                        out=gt,
                                        in_=gT_d[ht, :, ic * 512:(ic + 1) * 512],
                                    )
                                    nc.vector.tensor_mul(
                                        gated[:, ht, :], po[:, hh, :], gt)

                            if not do_pC:
                                pb = gst.tile([128, 512], f32, tag="pb",
                                              bufs=1, name="pb")
                                nc.vector.tensor_copy(
                                    pb, gated[:, 0, :].bitcast(f32))
                                nc.sync.dma_start(
                                    out=out.rearrange("(a p) m -> p a m", p=128)
                                    [:, 4 + ic, 0:512], in_=pb)

                        # ---- C: out2 = gatedT.T @ W_out; out = out2 * x ----
                        if do_pC:
                            for mc in range(2):
                                pos = p2ps.tile([128, 4, 512], f32, tag="acc",
                                                name="pos")
                                for hq in range(4):
                                    wo = wop.tile([128, 4, 512], f32r, tag="wo",
                                                  name="wo")
                                    for dh in range(2):
                                        eng = nc.sync if (hq + dh) % 2 else nc.gpsimd
                                        eng.dma_start(
                                            out=wo[:, dh * 2:(dh + 1) * 2, :],
                                            in_=wo_r[:, hq * 4 + dh * 2:
                                                     hq * 4 + (dh + 1) * 2,
                                                     mc * 512:(mc + 1) * 512],
                                        )
                                    for it in range(4):
                                        for h4 in range(4):
                                            nc.tensor.matmul(
                                                pos[:, it, :],
                                                gated[:, hq * 4 + h4,
                                                      it * 128:(it + 1) * 128],
                                                wo[:, h4, :],
                                                start=(hq == 0 and h4 == 0),
                                                stop=(hq == 3 and h4 == 3),
                                            )
                                for it in range(4):
                                    xo = xop.tile([128, 1024], f32, tag="xo",
                                                  name="xo")
                                    nc.sync.dma_start(
                                        out=xo, in_=xo_r[:, ic, it, :])
                                    ot = osp.tile([128, 512], f32, tag="ot",
                                                  name="ot")
                                    if obias:
                                        nc.vector.tensor_add(
                                            ot, pos[:, it, :],
                                            bo_t[:, mc * 512:(mc + 1) * 512])
                                        nc.vector.tensor_mul(
                                            ot, ot,
                                            xo[:, mc * 512:(mc + 1) * 512])
                                    else:
                                        nc.vector.tensor_mul(
                                            ot, pos[:, it, :],
                                            xo[:, mc * 512:(mc + 1) * 512])
                                    nc.sync.dma_start(
                                        out=out.rearrange(
                                            "(ic it p) m -> p ic it m",
                                            p=128, it=4)
                                        [:, ic, it, mc * 512:(mc + 1) * 512],
                                        in_=ot,
                                    )

            # anchor outputs for phase-subset timing builds (prevents DCE)
            if not (do_pA and do_pB and do_pC):
                tc.strict_bb_all_engine_barrier()
                with tc.tile_pool(name="probe", bufs=1) as prp:
                    if do_p1:
                        pt = prp.tile([128, 512], f32)
                        nc.sync.dma_start(
                            out=pt, in_=v_ag[0:128, 0:512].bitcast(f32))
                        nc.sync.dma_start(
                            out=out.rearrange("(a p) m -> p a m", p=128)
                            [:, 0, 0:512], in_=pt)

    nc.compile()
    return nc


# ---------------------------------------------------------------- runner ----

import time as _time

import jax
import jax.numpy as jnp
from jax.sharding import Mesh, NamedSharding, PartitionSpec
from jax.experimental.shard_map import shard_map

from concourse.bass2jax import _bass_exec_p, install_neuronx_cc_hook, partition_id_tensor


class SpmdRunner:
    def __init__(self, nc, n_cores=8):
        install_neuronx_cc_hook()
        self.nc = nc
        self.n_cores = n_cores
        partition_name = nc.partition_id_tensor.name if nc.partition_id_tensor else None
        in_names, out_names, out_avals, zero_outs = [], [], [], []
        for alloc in nc.m.functions[0].allocations:
            if not isinstance(alloc, mybir.MemoryLocationSet):
                continue
            name = alloc.memorylocations[0].name
            if alloc.kind == "ExternalInput":
                if name != partition_name:
                    in_names.append(name)
            elif alloc.kind == "ExternalOutput":
                shape = tuple(alloc.tensor_shape)
                dtype = mybir.dt.np(alloc.dtype)
                out_names.append(name)
                out_avals.append(jax.core.ShapedArray(shape, dtype))
                zero_outs.append(np.zeros(shape, dtype))
        self.in_names, self.out_names = in_names, out_names
        self.out_avals, self.zero_outs = out_avals, zero_outs
        n_params = len(in_names)
        all_names = in_names + out_names
        if partition_name is not None:
            all_names = all_names + [partition_name]

        def _body(*args):
            operands = list(args)
            if partition_name is not None:
                operands.append(partition_id_tensor())
            outs = _bass_exec_p.bind(
                *operands,
                out_avals=tuple(out_avals),
                in_names=tuple(all_names),
                out_names=tuple(out_names),
                lowering_input_output_aliases=(),
                sim_require_finite=True,
                sim_require_nnan=True,
                nc=nc,
            )
            return tuple(outs)

        devices = jax.devices()[:n_cores]
        self.mesh = Mesh(np.asarray(devices), ("core",))
        in_specs = (PartitionSpec("core"),) * (n_params + len(out_names))
        out_specs = (PartitionSpec("core"),) * len(out_names)
        self.sharded = jax.jit(
            shard_map(_body, mesh=self.mesh, in_specs=in_specs,
                      out_specs=out_specs, check_rep=False),
            keep_unused=True,
        )

    def stage_inputs(self, in_maps):
        n = self.n_cores
        concat = [
            np.concatenate([np.asarray(in_maps[c][name]) for c in range(n)], axis=0)
            for name in self.in_names
        ]
        concat += [np.zeros((n * z.shape[0], *z.shape[1:]), z.dtype)
                   for z in self.zero_outs]
        sharding = NamedSharding(self.mesh, PartitionSpec("core"))
        return [jax.device_put(a, sharding) for a in concat]

    def run(self, staged):
        outs = self.sharded(*staged)
        jax.block_until_ready(outs)
        return outs

    def run_numpy(self, staged):
        outs = self.run(staged)
        n = self.n_cores
        return [
            {name: np.asarray(outs[i]).reshape(n, *self.out_avals[i].shape)[c]
             for i, name in enumerate(self.out_names)}
            for c in range(n)
        ]


# ------------------------------------------------------------- host side ----

_CACHE = {}


def _get_runner(reps, vbias, obias):
    key = (reps, vbias, obias)
    if key not in _CACHE:
        nc = _build_nc(reps=reps, vbias=vbias, obias=obias)
        _CACHE[key] = SpmdRunner(nc, NC)
    return _CACHE[key]


def _pad2(v):
    o = np.zeros((2, 128), np.float32)
    o[0] = v[:128]
    o[1, :QK - 128] = v[128:QK]
    return o


def make_in_maps(x, W_hidden, b_hidden, W_qk, b_qk, gamma, beta, W_out, b_out):
    x = np.ascontiguousarray(np.asarray(x, np.float32))
    scale = 1.0 / np.sqrt(np.float32(D))
    gq = _pad2(np.asarray(gamma[0], np.float32) * scale)
    bq = _pad2(np.asarray(beta[0], np.float32) * scale)
    gk = _pad2(np.asarray(gamma[1], np.float32))
    bk = _pad2(np.asarray(beta[1], np.float32))
    bqk = _pad2(np.asarray(b_qk, np.float32))
    bg = np.ascontiguousarray(
        np.asarray(b_hidden[H:], np.float32).reshape(HT, 128))
    W_hidden = np.ascontiguousarray(np.asarray(W_hidden, np.float32))
    W_qk = np.ascontiguousarray(np.asarray(W_qk, np.float32))
    W_out = np.ascontiguousarray(np.asarray(W_out, np.float32))
    bv = np.asarray(b_hidden[:H], np.float32)
    bo = np.asarray(b_out, np.float32)
    vbias = bool(np.any(bv))
    obias = bool(np.any(bo))

    xT = np.ascontiguousarray(x.T)
    in_maps = []
    for c in range(NC):
        m = {
            "xT_own": np.ascontiguousarray(xT[:, c * R:(c + 1) * R]),
            "x_own": x[c * R:(c + 1) * R],
            "w_h": W_hidden,
            "w_qk": W_qk,
            "w_out": W_out,
            "gq": gq, "bq": bq, "gk": gk, "bk": bk, "bqk": bqk, "bg": bg,
        }
        if vbias:
            m["bv"] = bv
        if obias:
            m["bo"] = bo
        in_maps.append(m)
    return in_maps, vbias, obias


def kernel(x, W_hidden, b_hidden, W_qk, b_qk, gamma, beta, W_out, b_out):
    in_maps, vbias, obias = make_in_maps(
        x, W_hidden, b_hidden, W_qk, b_qk, gamma, beta, W_out, b_out)
    runner = _get_runner(1, vbias, obias)
    staged = runner.stage_inputs(in_maps)
    results = runner.run_numpy(staged)
    return np.concatenate([results[c]["out"] for c in range(NC)], axis=0)


# revision 8
# speedup vs baseline: 1.4866x; 1.4866x over previous
"""GAU (Gated Attention Unit) Trainium2 Bass kernel, 8-core sequence-parallel.

Reference computation (all fp32):
    hid  = silu(x @ W_hidden + b_hidden);  v, gate = split(hid, 2)
    qk   = silu(x @ W_qk + b_qk)
    q    = qk * gamma[0] + beta[0];  k = qk * gamma[1] + beta[1]
    attn = relu((q @ k.T) / sqrt(dim))^2
    out  = ((attn @ v) * gate) @ W_out + b_out
    return out * x

Sharding (v2): rows (N=8192) split across 8 cores, 1024 rows each. Each core
computes k / v / q / gate for its OWN rows only, then AllGathers k and v
across the 8 cores. The collectives run on TOPSP+SDMA (separate silicon), so
they overlap with PE compute: AG(k) hides under the v/gate matmuls, AG(v)
hides under the gate matmuls + sim generation. This cuts per-core PE work
~40% vs replicating the k/v compute on every core.

All matmuls run as float32r (TF32-like, full PE rate at free-dim >= 256).
x is transposed on the host (xT_own input) so the kernel needs no transposes.
"""

import numpy as np

import concourse.bass as bass
import concourse.mybir as mybir
import concourse.tile as tile
from concourse import bacc

N = 8192          # total rows
D = 1024          # model dim
QK = 200          # qk dim
H = 2048          # hidden (v/gate) dim
NC = 8            # cores
R = N // NC       # rows per core
DT = D // 128     # d-tiles
HT = H // 128     # h-tiles
IC = R // 512     # i-chunks per core (own j-groups of 512)

f32 = mybir.dt.float32
f32r = mybir.dt.float32r
bf16 = mybir.dt.bfloat16
ACT = mybir.ActivationFunctionType
ALU = mybir.AluOpType
RG = [list(range(NC))]


def _build_nc(reps=1, vbias=False, obias=False,
              do_p1=True, do_pA=True, do_pB=True, do_pC=True):
    nc = bacc.Bacc("TRN2", target_bir_lowering=False, debug=False,
                   num_devices=NC)

    xT_own = nc.dram_tensor("xT_own", [D, R], f32r, kind="ExternalInput").ap()
    x_own = nc.dram_tensor("x_own", [R, D], f32, kind="ExternalInput").ap()
    w_h = nc.dram_tensor("w_h", [D, 2 * H], f32r, kind="ExternalInput").ap()
    w_qk = nc.dram_tensor("w_qk", [D, QK], f32r, kind="ExternalInput").ap()
    w_out = nc.dram_tensor("w_out", [H, D], f32r, kind="ExternalInput").ap()
    # per-c scalars, padded 200 -> [2, 128]
    gq = nc.dram_tensor("gq", [2, 128], f32, kind="ExternalInput").ap()
    bq = nc.dram_tensor("bq", [2, 128], f32, kind="ExternalInput").ap()
    gk = nc.dram_tensor("gk", [2, 128], f32, kind="ExternalInput").ap()
    bk = nc.dram_tensor("bk", [2, 128], f32, kind="ExternalInput").ap()
    bqk = nc.dram_tensor("bqk", [2, 128], f32, kind="ExternalInput").ap()
    bg = nc.dram_tensor("bg", [HT, 128], f32, kind="ExternalInput").ap()
    if vbias:
        bv = nc.dram_tensor("bv", [H], f32, kind="ExternalInput").ap()
    if obias:
        bo = nc.dram_tensor("bo", [D], f32, kind="ExternalInput").ap()
    out = nc.dram_tensor("out", [R, D], f32, kind="ExternalOutput").ap()

    with tile.TileContext(nc) as tc:
        with (
            tc.tile_pool(name="pers", bufs=1) as pers,
            tc.tile_pool(name="dram", bufs=1, space="DRAM") as dpool,
        ):
            # persistent small tiles
            gq_t = pers.tile([128, 2], f32)
            bq_t = pers.tile([128, 2], f32)
            gk_t = pers.tile([128, 2], f32)
            bk_t = pers.tile([128, 2], f32)
            bqk_t = pers.tile([128, 2], f32)
            bg_t = pers.tile([128, HT], f32)
            nc.sync.dma_start(out=gq_t, in_=gq.rearrange("ct c -> c ct"))
            nc.sync.dma_start(out=bq_t, in_=bq.rearrange("ct c -> c ct"))
            nc.sync.dma_start(out=gk_t, in_=gk.rearrange("ct c -> c ct"))
            nc.sync.dma_start(out=bk_t, in_=bk.rearrange("ct c -> c ct"))
            nc.sync.dma_start(out=bqk_t, in_=bqk.rearrange("ct c -> c ct"))
            nc.sync.dma_start(out=bg_t, in_=bg.rearrange("ht c -> c ht"))
            if vbias:
                bv_t = pers.tile([128, H], f32)
                nc.sync.dma_start(
                    out=bv_t,
                    in_=bass.AP(tensor=bv.tensor, offset=bv.offset,
                                ap=[[0, 128]] + list(bv.ap)),
                )
            if obias:
                bo_t = pers.tile([128, D], f32)
                nc.sync.dma_start(
                    out=bo_t,
                    in_=bass.AP(tensor=bo.tensor, offset=bo.offset,
                                ap=[[0, 128]] + list(bo.ap)),
                )

            # DRAM scratch: own-row slices (AG inputs, Local) and the
            # gathered full tensors (AG outputs, Shared)
            kT_own = dpool.tile([2, 128, R], f32r, tag="kT_own")
            v_own = dpool.tile([R, H], bf16, tag="v_own")
            # Shared (AG output) tiles allow only one writer inst each ->
            # one pair per timing rep
            kT_ags = [dpool.tile([NC, 2, 128, R], f32r, tag=f"kT_ag{r}",
                                 name=f"kT_ag{r}", addr_space="Shared")
                      for r in range(reps)]
            v_ags = [dpool.tile([N, H], bf16, tag=f"v_ag{r}",
                                name=f"v_ag{r}", addr_space="Shared")
                     for r in range(reps)]
            gT_d = dpool.tile([HT, 128, R], f32, tag="gT_d")
            # qT lives in SBUF for the whole kernel (1 MB)
            qT_s = pers.tile([128, 2, R], f32r, tag="qT_s", name="qT_s")

            xT_r = xT_own.rearrange("(dt p) (jg j) -> p dt jg j", p=128, j=512)
            wh_r = w_h.rearrange("(dt p) h -> p dt h", p=128)
            wqk_r = w_qk.rearrange("(dt p) c -> p dt c", p=128)
            wo_r = w_out.rearrange("(ht p) m -> p ht m", p=128)
            xo_r = x_own.rearrange("(ic it p) m -> p ic it m", p=128, it=4)

            for rep in range(reps):
                if rep:
                    # full barrier between timing reps so SBUF/PSUM region
                    # reuse across the rep boundary is strictly ordered
                    tc.strict_bb_all_engine_barrier()
                kT_ag = kT_ags[rep]
                v_ag = v_ags[rep]
                # ============ phase 1: own-row k, q, v, gate + AGs ============
                if not do_p1:
                    pass
                else:
                 with (
                    tc.tile_pool(name="whp", bufs=1) as whp,
                    tc.tile_pool(name="xgp", bufs=1) as xgp,
                    tc.tile_pool(name="st1", bufs=(2 if vbias else 3)) as st1,
                    tc.tile_pool(name="ps_qk", bufs=2, space="PSUM") as ps_qk,
                    tc.tile_pool(name="ps_v", bufs=2, space="PSUM") as ps_v,
                    tc.tile_pool(name="ps_g", bufs=2, space="PSUM") as ps_g,
                ):
                    wqk_t = whp.tile([128, DT, QK], f32r, tag="wqk")
                    nc.sync.dma_start(out=wqk_t, in_=wqk_r)
                    # both own j-groups of x stay in SBUF all of phase 1
                    xg = xgp.tile([128, DT, IC, 512], f32r, tag="xg")
                    for jg in range(IC):
                        for dh in range(2):
                            eng = nc.sync if (jg + dh) % 2 else nc.gpsimd
                            eng.dma_start(
                                out=xg[:, dh * 4:(dh + 1) * 4, jg, :],
                                in_=xT_r[:, dh * 4:(dh + 1) * 4, jg, :])

                    # ---- qk -> k (own rows) + q (own rows) ----
                    for jg in range(IC):
                        for ct in range(2):
                            cw = 128 if ct == 0 else QK - 128
                            pq = ps_qk.tile([128, 512], f32)
                            for dt in range(DT):
                                nc.tensor.matmul(
                                    pq[:cw],
                                    wqk_t[:, dt, ct * 128:ct * 128 + cw],
                                    xg[:, dt, jg, :],
                                    start=(dt == 0),
                                    stop=(dt == DT - 1),
                                )
                            sil = st1.tile([128, 512], f32, tag="sil")
                            nc.scalar.activation(
                                sil[:cw], pq[:cw], ACT.Silu,
                                bias=bqk_t[:cw, ct:ct + 1],
                            )
                            kt = st1.tile([128, 512], f32r, tag="kt")
                            nc.vector.tensor_scalar(
                                out=kt[:cw], in0=sil[:cw],
                                scalar1=gk_t[:cw, ct:ct + 1],
                                scalar2=bk_t[:cw, ct:ct + 1],
                                op0=ALU.mult, op1=ALU.add,
                            )
                            nc.sync.dma_start(
                                out=kT_own[ct, 0:cw, jg * 512:(jg + 1) * 512],
                                in_=kt[:cw],
                            )
                            nc.vector.tensor_scalar(
                                out=qT_s[:cw, ct, jg * 512:(jg + 1) * 512],
                                in0=sil[:cw],
                                scalar1=gq_t[:cw, ct:ct + 1],
                                scalar2=bq_t[:cw, ct:ct + 1],
                                op0=ALU.mult, op1=ALU.add,
                            )
                    # k slice done -> gather all k while v/gate compute
                    nc.gpsimd.collective_compute(
                        "AllGather", ALU.bypass, replica_groups=RG,
                        ins=[kT_own.opt()], outs=[kT_ag.opt()],
                    )

                    wh_t = whp.tile([128, DT, 2 * H], f32r, tag="wh")
                    for dt in range(DT):
                        eng = nc.sync if dt % 2 else nc.scalar
                        eng.dma_start(out=wh_t[:, dt, :], in_=wh_r[:, dt, :])

                    # ---- v (own rows, row-major for the AG) ----
                    for jg in range(IC):
                        for jt in range(4):
                            for hc in range(4):
                                pv = ps_v.tile([128, 512], f32)
                                for dt in range(DT):
                                    nc.tensor.matmul(
                                        pv,
                                        xg[:, dt, jg, jt * 128:(jt + 1) * 128],
                                        wh_t[:, dt, hc * 512:(hc + 1) * 512],
                                        start=(dt == 0),
                                        stop=(dt == DT - 1),
                                    )
                                vt = st1.tile([128, 512], bf16, tag="vt")
                                if vbias:
                                    tmp = st1.tile([128, 512], f32, tag="vtmp")
                                    nc.vector.tensor_add(
                                        tmp, pv, bv_t[:, hc * 512:(hc + 1) * 512])
                                    nc.scalar.activation(vt, tmp, ACT.Silu)
                                else:
                                    nc.scalar.activation(vt, pv, ACT.Silu)
                                veng = nc.sync if (jt + hc) % 2 else nc.scalar
                                veng.dma_start(
                                    out=v_own[(jg * 4 + jt) * 128:
                                              (jg * 4 + jt + 1) * 128,
                                              hc * 512:(hc + 1) * 512],
                                    in_=vt,
                                )
                    # v slice done -> gather all v while gate computes
                    nc.gpsimd.collective_compute(
                        "AllGather", ALU.bypass, replica_groups=RG,
                        ins=[v_own.opt()], outs=[v_ag.opt()],
                    )

                    # ---- gateT (own rows) ----
                    for jg in range(IC):
                        for ht in range(HT):
                            pg = ps_g.tile([128, 512], f32)
                            for dt in range(DT):
                                nc.tensor.matmul(
                                    pg,
                                    wh_t[:, dt, H + ht * 128:H + (ht + 1) * 128],
                                    xg[:, dt, jg, :],
                                    start=(dt == 0),
                                    stop=(dt == DT - 1),
                                )
                            gt = st1.tile([128, 512], f32, tag="gt")
                            nc.scalar.activation(
                                gt, pg, ACT.Silu, bias=bg_t[:, ht:ht + 1])
                            nc.sync.dma_start(
                                out=gT_d[ht, :, jg * 512:(jg + 1) * 512],
                                in_=gt,
                            )

                # ================= phase 2: attention per i-chunk =================
                with (
                    tc.tile_pool(name="p2sb", bufs=1) as p2sb,
                    tc.tile_pool(name="kqp", bufs=2) as kqp,
                    tc.tile_pool(name="vst", bufs=5) as vst,
                    tc.tile_pool(name="gst", bufs=2) as gst,
                    tc.tile_pool(name="wop", bufs=2) as wop,
                    tc.tile_pool(name="xop", bufs=1) as xop,
                    tc.tile_pool(name="ost", bufs=1) as osp,
                    tc.tile_pool(name="p2ps", bufs=1, space="PSUM") as p2ps,
                ):
                    # PSUM layout: tag "sim" = 2 banks (A), tag "acc" = 6 banks
                    # (B out1T h-groups of 6/6/4, C out2 4 i-tiles) -> 8 total,
                    # static, so A/B/C of consecutive i-chunks overlap freely.
                    for ic in range(IC):
                        attn = p2sb.tile([128, N // 128, 512], bf16,
                                         tag="attn", name="attn")
                        gated = p2sb.tile([128, HT, 512], f32r,
                                          tag="gated", name="gated")

                        # ---- A: attn[j, i-chunk] = relu(k.T q)^2 ----
                        if do_pA:
                            q_sb = qT_s[:, :, ic * 512:(ic + 1) * 512]
                            for jg in range(N // 512):
                                kt_sb = kqp.tile([128, 2, 512], f32r,
                                                 tag="kt_sb", name="kt_sb")
                                keng = nc.sync if jg % 2 else nc.gpsimd
                                keng.dma_start(
                                    out=kt_sb,
                                    in_=kT_ag[jg // 2, :, :,
                                              (jg % 2) * 512:(jg % 2 + 1) * 512]
                                    .rearrange("ct c j -> c ct j"),
                                )
                                for j4 in range(4):
                                    jt = jg * 4 + j4
                                    pss = p2ps.tile([128, 512], f32, tag="sim",
                                                    bufs=2, name="pss")
                                    nc.tensor.matmul(
                                        pss, kt_sb[:, 0, j4 * 128:(j4 + 1) * 128],
                                        q_sb[:, 0, :], start=True, stop=False)
                                    nc.tensor.matmul(
                                        pss, kt_sb[0:QK - 128, 1,
                                                   j4 * 128:(j4 + 1) * 128],
                                        q_sb[0:QK - 128, 1, :],
                                        start=False, stop=True)
                                    rel = kqp.tile([128, 512], f32,
                                                   tag="rel", bufs=2,
                                                   name="rel")
                                    nc.scalar.activation(rel, pss, ACT.Relu)
                                    nc.vector.tensor_mul(
                                        attn[:, jt, :], rel, rel)

                            if not (do_pB and do_pC):
                                pa = kqp.tile([128, 512], f32, tag="pa",
                                              bufs=1, name="pa")
                                nc.vector.tensor_copy(pa, attn[:, 0, :])
                                nc.sync.dma_start(
                                    out=out.rearrange("(a p) m -> p a m", p=128)
                                    [:, 1 + ic, 0:512], in_=pa)

                        # ---- B: out1T[h, i-chunk] = v-lhsT @ attn; * gateT ----
                        if do_pB:
                            for h0, nht in ((0, 6), (6, 6), (12, 4)):
                                po = p2ps.tile([128, nht, 512], f32, tag="acc",
                                               name="po")
                                for jt in range(N // 128):
                                    vt = vst.tile([128, nht * 128], bf16,
                                                  tag="vt", name="vt")
                                    half = nht * 64
                                    for dh in range(2):
                                        eng = nc.sync if (jt + dh) % 2 else nc.gpsimd
                                        eng.dma_start(
                                            out=vt[:, dh * half:(dh + 1) * half],
                                            in_=v_ag[jt * 128:(jt + 1) * 128,
                                                     h0 * 128 + dh * half:
                                                     h0 * 128 + (dh + 1) * half],
                                        )
                                    for hh in range(nht):
                                        nc.tensor.matmul(
                                            po[:, hh, :],
                                            vt[:, hh * 128:(hh + 1) * 128],
                                            attn[:, jt, :],
                                            start=(jt == 0),
                                            stop=(jt == N // 128 - 1),
                                        )
                                for hh in range(nht):
                                    ht = h0 + hh
                                    gt = gst.tile([128, 512], f32, tag="gt",
                                                  name="gt")
                                    geng = nc.sync if ht % 2 else nc.gpsimd
                                    geng.dma_start(
                                        out=gt,
                                        in_=gT_d[ht, :, ic * 512:(ic + 1) * 512],
                                    )
                                    nc.vector.tensor_mul(
                                        gated[:, ht, :], po[:, hh, :], gt)

                            if not do_pC:
                                pb = gst.tile([128, 512], f32, tag="pb",
                                              bufs=1, name="pb")
                                nc.vector.tensor_copy(
                                    pb, gated[:, 0, :].bitcast(f32))
                                nc.sync.dma_start(
                                    out=out.rearrange("(a p) m -> p a m", p=128)
                                    [:, 4 + ic, 0:512], in_=pb)

                        # ---- C: out2 = gatedT.T @ W_out; out = out2 * x ----
                        if do_pC:
                            for mc in range(2):
                                pos = p2ps.tile([128, 4, 512], f32, tag="acc",
                                                name="pos")
                                for hq in range(4):
                                    wo = wop.tile([128, 4, 512], f32r, tag="wo",
                                                  name="wo")
                                    for dh in range(2):
                                        eng = nc.sync if (hq + dh) % 2 else nc.gpsimd
                                        eng.dma_start(
                                            out=wo[:, dh * 2:(dh + 1) * 2, :],
                                            in_=wo_r[:, hq * 4 + dh * 2:
                                                     hq * 4 + (dh + 1) * 2,
                                                     mc * 512:(mc + 1) * 512],
                                        )
                                    for it in range(4):
                                        for h4 in range(4):
                                            nc.tensor.matmul(
                                                pos[:, it, :],
                                                gated[:, hq * 4 + h4,
                                                      it * 128:(it + 1) * 128],
                                                wo[:, h4, :],
                                                start=(hq == 0 and h4 == 0),
                                                stop=(hq == 3 and h4 == 3),
                                            )
                                for it in range(4):
                                    xo = xop.tile([128, 1024], f32, tag="xo",
                                                  name="xo")
                                    nc.sync.dma_start(
                                        out=xo, in_=xo_r[:, ic, it, :])
                                    ot = osp.tile([128, 512], f32, tag="ot",
                                                  name="ot")
                                    if obias:
                                        nc.vector.tensor_add(
                                            ot, pos[:, it, :],
                                            bo_t[:, mc * 512:(mc + 1) * 512])
                                        nc.vector.tensor_mul(
                                            ot, ot,
                                            xo[:, mc * 512:(mc + 1) * 512])
                                    else:
                                        nc.vector.tensor_mul(
                                            ot, pos[:, it, :],
                                            xo[:, mc * 512:(mc + 1) * 512])
                                    nc.sync.dma_start(
                                        out=out.rearrange(
                                            "(ic it p) m -> p ic it m",
                                            p=128, it=4)
                                        [:, ic, it, mc * 512:(mc + 1) * 512],
                                        in_=ot,
                                    )

            # anchor outputs for phase-subset timing builds (prevents DCE)
            if not (do_pA and do_pB and do_pC):
                tc.strict_bb_all_engine_barrier()
                with tc.tile_pool(name="probe", bufs=1) as prp:
                    if do_p1:
                        pt = prp.tile([128, 512], f32)
                        nc.sync.dma_start(
                            out=pt, in_=v_ag[0:128, 0:512].bitcast(f32))
                        nc.sync.dma_start(
                            out=out.rearrange("(a p) m -> p a m", p=128)
                            [:, 0, 0:512], in_=pt)

    nc.compile()
    return nc


# ---------------------------------------------------------------- runner ----

import time as _time

import jax
import jax.numpy as jnp
from jax.sharding import Mesh, NamedSharding, PartitionSpec
from jax.experimental.shard_map import shard_map

from concourse.bass2jax import _bass_exec_p, install_neuronx_cc_hook, partition_id_tensor


class SpmdRunner:
    def __init__(self, nc, n_cores=8):
        install_neuronx_cc_hook()
        self.nc = nc
        self.n_cores = n_cores
        partition_name = nc.partition_id_tensor.name if nc.partition_id_tensor else None
        in_names, out_names, out_avals, zero_outs = [], [], [], []
        for alloc in nc.m.functions[0].allocations:
            if not isinstance(alloc, mybir.MemoryLocationSet):
                continue
            name = alloc.memorylocations[0].name
            if alloc.kind == "ExternalInput":
                if name != partition_name:
                    in_names.append(name)
            elif alloc.kind == "ExternalOutput":
                shape = tuple(alloc.tensor_shape)
                dtype = mybir.dt.np(alloc.dtype)
                out_names.append(name)
                out_avals.append(jax.core.ShapedArray(shape, dtype))
                zero_outs.append(np.zeros(shape, dtype))
        self.in_names, self.out_names = in_names, out_names
        self.out_avals, self.zero_outs = out_avals, zero_outs
        n_params = len(in_names)
        all_names = in_names + out_names
        if partition_name is not None:
            all_names = all_names + [partition_name]

        def _body(*args):
            operands = list(args)
            if partition_name is not None:
                operands.append(partition_id_tensor())
            outs = _bass_exec_p.bind(
                *operands,
                out_avals=tuple(out_avals),
                in_names=tuple(all_names),
                out_names=tuple(out_names),
                lowering_input_output_aliases=(),
                sim_require_finite=True,
                sim_require_nnan=True,
                nc=nc,
            )
            return tuple(outs)

        devices = jax.devices()[:n_cores]
        self.mesh = Mesh(np.asarray(devices), ("core",))
        in_specs = (PartitionSpec("core"),) * (n_params + len(out_names))
        out_specs = (PartitionSpec("core"),) * len(out_names)
        self.sharded = jax.jit(
            shard_map(_body, mesh=self.mesh, in_specs=in_specs,
                      out_specs=out_specs, check_rep=False),
            keep_unused=True,
        )

    def stage_inputs(self, in_maps):
        n = self.n_cores
        concat = [
            np.concatenate([np.asarray(in_maps[c][name]) for c in range(n)], axis=0)
            for name in self.in_names
        ]
        concat += [np.zeros((n * z.shape[0], *z.shape[1:]), z.dtype)
                   for z in self.zero_outs]
        sharding = NamedSharding(self.mesh, PartitionSpec("core"))
        return [jax.device_put(a, sharding) for a in concat]

    def run(self, staged):
        outs = self.sharded(*staged)
        jax.block_until_ready(outs)
        return outs

    def run_numpy(self, staged):
        outs = self.run(staged)
        n = self.n_cores
        return [
            {name: np.asarray(outs[i]).reshape(n, *self.out_avals[i].shape)[c]
             for i, name in enumerate(self.out_names)}
            for c in range(n)
        ]


# ------------------------------------------------------------- host side ----

_CACHE = {}


def _get_runner(reps, vbias, obias):
    key = (reps, vbias, obias)
    if key not in _CACHE:
        nc = _build_nc(reps=reps, vbias=vbias, obias=obias)
        _CACHE[key] = SpmdRunner(nc, NC)
    return _CACHE[key]


def _pad2(v):
    o = np.zeros((2, 128), np.float32)
    o[0] = v[:128]
    o[1, :QK - 128] = v[128:QK]
    return o


def make_in_maps(x, W_hidden, b_hidden, W_qk, b_qk, gamma, beta, W_out, b_out):
    x = np.ascontiguousarray(np.asarray(x, np.float32))
    scale = 1.0 / np.sqrt(np.float32(D))
    gq = _pad2(np.asarray(gamma[0], np.float32) * scale)
    bq = _pad2(np.asarray(beta[0], np.float32) * scale)
    gk = _pad2(np.asarray(gamma[1], np.float32))
    bk = _pad2(np.asarray(beta[1], np.float32))
    bqk = _pad2(np.asarray(b_qk, np.float32))
    bg = np.ascontiguousarray(
        np.asarray(b_hidden[H:], np.float32).reshape(HT, 128))
    W_hidden = np.ascontiguousarray(np.asarray(W_hidden, np.float32))
    W_qk = np.ascontiguousarray(np.asarray(W_qk, np.float32))
    W_out = np.ascontiguousarray(np.asarray(W_out, np.float32))
    bv = np.asarray(b_hidden[:H], np.float32)
    bo = np.asarray(b_out, np.float32)
    vbias = bool(np.any(bv))
    obias = bool(np.any(bo))

    xT = np.ascontiguousarray(x.T)
    in_maps = []
    for c in range(NC):
        m = {
            "xT_own": np.ascontiguousarray(xT[:, c * R:(c + 1) * R]),
            "x_own": x[c * R:(c + 1) * R],
            "w_h": W_hidden,
            "w_qk": W_qk,
            "w_out": W_out,
            "gq": gq, "bq": bq, "gk": gk, "bk": bk, "bqk": bqk, "bg": bg,
        }
        if vbias:
            m["bv"] = bv
        if obias:
            m["bo"] = bo
        in_maps.append(m)
    return in_maps, vbias, obias


def kernel(x, W_hidden, b_hidden, W_qk, b_qk, gamma, beta, W_out, b_out):
    in_maps, vbias, obias = make_in_maps(
        x, W_hidden, b_hidden, W_qk, b_qk, gamma, beta, W_out, b_out)
    runner = _get_runner(1, vbias, obias)
    staged = runner.stage_inputs(in_maps)
    results = runner.run_numpy(staged)
    return np.concatenate([results[c]["out"] for c in range(NC)], axis=0)


# revision 11
# speedup vs baseline: 1.8984x; 1.2770x over previous
"""GAU (Gated Attention Unit) Trainium2 Bass kernel, 8-core sequence-parallel.

Reference computation (all fp32):
    hid  = silu(x @ W_hidden + b_hidden);  v, gate = split(hid, 2)
    qk   = silu(x @ W_qk + b_qk)
    q    = qk * gamma[0] + beta[0];  k = qk * gamma[1] + beta[1]
    attn = relu((q @ k.T) / sqrt(dim))^2
    out  = ((attn @ v) * gate) @ W_out + b_out
    return out * x

Sharding (v3): rows (N=8192) split across 8 cores, 1024 rows each. Each core
computes k / v / q / gate for its OWN rows only, then AllGathers k and v
across the 8 cores. The collectives run on TOPSP+SDMA (separate silicon), so
they overlap with PE compute. Phase-1 order is chosen so the long AG(v)
(~160us wall) is triggered as early as possible and hides under the gate
matmuls + attn generation: qk -> AG(k) -> v -> AG(v) -> gate -> attn.

Phase 2 computes attn for BOTH 512-row i-chunks first (attn SBUF tile covers
all 1024 own rows), then one attn@v pass reads each v tile once, then the
W_out contraction. All PSUM goes through a single "acc" tag of 4-bank slots,
double-buffered (8 banks total), so group boundaries overlap.

Matmuls are f32r except attn@v and W_out (bf16 operands). W_hidden / W_out
are staged as bf16 on the host (halves weight DMA; rel err stays ~1e-3).
"""

import numpy as np

import concourse.bass as bass
import concourse.mybir as mybir
import concourse.tile as tile
from concourse import bacc

N = 8192          # total rows
D = 1024          # model dim
QK = 200          # qk dim
H = 2048          # hidden (v/gate) dim
NC = 8            # cores
R = N // NC       # rows per core
DT = D // 128     # d-tiles
HT = H // 128     # h-tiles
IC = R // 512     # i-chunks per core (own j-groups of 512)
JT = N // 128     # total j-tiles

f32 = mybir.dt.float32
f32r = mybir.dt.float32r
bf16 = mybir.dt.bfloat16
ACT = mybir.ActivationFunctionType
ALU = mybir.AluOpType
RG = [list(range(NC))]


def _build_nc(reps=1, vbias=False, obias=False,
              do_p1=True, do_pA=True, do_pB=True, do_pC=True):
    nc = bacc.Bacc("TRN2", target_bir_lowering=False, debug=False,
                   num_devices=NC)

    xT_own = nc.dram_tensor("xT_own", [D, R], bf16, kind="ExternalInput").ap()
    x_own = nc.dram_tensor("x_own", [R, D], f32, kind="ExternalInput").ap()
    w_h = nc.dram_tensor("w_h", [D, 2 * H], bf16, kind="ExternalInput").ap()
    w_qk = nc.dram_tensor("w_qk", [D, QK], bf16, kind="ExternalInput").ap()
    w_out = nc.dram_tensor("w_out", [H, D], bf16, kind="ExternalInput").ap()
    # per-c scalars, padded 200 -> [2, 128]
    gq = nc.dram_tensor("gq", [2, 128], f32, kind="ExternalInput").ap()
    bq = nc.dram_tensor("bq", [2, 128], f32, kind="ExternalInput").ap()
    gk = nc.dram_tensor("gk", [2, 128], f32, kind="ExternalInput").ap()
    bk = nc.dram_tensor("bk", [2, 128], f32, kind="ExternalInput").ap()
    bqk = nc.dram_tensor("bqk", [2, 128], f32, kind="ExternalInput").ap()
    bg = nc.dram_tensor("bg", [HT, 128], f32, kind="ExternalInput").ap()
    if vbias:
        bv = nc.dram_tensor("bv", [H], f32, kind="ExternalInput").ap()
    if obias:
        bo = nc.dram_tensor("bo", [D], f32, kind="ExternalInput").ap()
    out = nc.dram_tensor("out", [R, D], f32, kind="ExternalOutput").ap()

    with tile.TileContext(nc) as tc:
        with (
            tc.tile_pool(name="pers", bufs=1) as pers,
            tc.tile_pool(name="dram", bufs=1, space="DRAM") as dpool,
        ):
            # persistent small tiles
            gq_t = pers.tile([128, 2], f32)
            bq_t = pers.tile([128, 2], f32)
            gk_t = pers.tile([128, 2], f32)
            bk_t = pers.tile([128, 2], f32)
            bqk_t = pers.tile([128, 2], f32)
            bg_t = pers.tile([128, HT], f32)
            nc.sync.dma_start(out=gq_t, in_=gq.rearrange("ct c -> c ct"))
            nc.sync.dma_start(out=bq_t, in_=bq.rearrange("ct c -> c ct"))
            nc.sync.dma_start(out=gk_t, in_=gk.rearrange("ct c -> c ct"))
            nc.sync.dma_start(out=bk_t, in_=bk.rearrange("ct c -> c ct"))
            nc.sync.dma_start(out=bqk_t, in_=bqk.rearrange("ct c -> c ct"))
            nc.sync.dma_start(out=bg_t, in_=bg.rearrange("ht c -> c ht"))
            if vbias:
                bv_t = pers.tile([128, H], f32)
                nc.sync.dma_start(
                    out=bv_t,
                    in_=bass.AP(tensor=bv.tensor, offset=bv.offset,
                                ap=[[0, 128]] + list(bv.ap)),
                )
            if obias:
                bo_t = pers.tile([128, D], f32)
                nc.sync.dma_start(
                    out=bo_t,
                    in_=bass.AP(tensor=bo.tensor, offset=bo.offset,
                                ap=[[0, 128]] + list(bo.ap)),
                )

            # DRAM scratch: own-row slices (AG inputs, Local) and the
            # gathered full tensors (AG outputs, Shared; single-writer each,
            # so one pair per timing rep)
            kT_own = dpool.tile([2, 128, R], f32r, tag="kT_own")
            v_own = dpool.tile([R, H], bf16, tag="v_own")
            kT_ags = [dpool.tile([NC, 2, 128, R], f32r, tag=f"kT_ag{r}",
                                 name=f"kT_ag{r}", addr_space="Shared")
                      for r in range(reps)]
            v_ags = [dpool.tile([N, H], bf16, tag=f"v_ag{r}",
                                name=f"v_ag{r}", addr_space="Shared")
                     for r in range(reps)]
            gT_d = dpool.tile([HT, 128, R], bf16, tag="gT_d")
            # qT lives in SBUF for the whole kernel (1 MB)
            qT_s = pers.tile([128, 2, R], f32r, tag="qT_s", name="qT_s")

            xT_r = xT_own.rearrange("(dt p) (jg j) -> p dt jg j", p=128, j=512)
            wh_r = w_h.rearrange("(dt p) h -> p dt h", p=128)
            wqk_r = w_qk.rearrange("(dt p) c -> p dt c", p=128)
            wo_r = w_out.rearrange("(ht p) m -> p ht m", p=128)
            xo_r = x_own.rearrange("(it p) m -> p it m", p=128)

            for rep in range(reps):
                if rep:
                    # full barrier between timing reps so SBUF/PSUM region
                    # reuse across the rep boundary is strictly ordered
                    tc.strict_bb_all_engine_barrier()
                kT_ag = kT_ags[rep]
                v_ag = v_ags[rep]
                # ============ phase 1: own-row k, q, v, gate + AGs ============
                if not do_p1:
                    pass
                else:
                 with (
                    tc.tile_pool(name="whp", bufs=1) as whp,
                    tc.tile_pool(name="xgp", bufs=1) as xgp,
                    tc.tile_pool(name="st1", bufs=(2 if vbias else 3)) as st1,
                    tc.tile_pool(name="ps_qk", bufs=2, space="PSUM") as ps_qk,
                    tc.tile_pool(name="ps_v", bufs=2, space="PSUM") as ps_v,
                    tc.tile_pool(name="ps_g", bufs=2, space="PSUM") as ps_g,
                ):
                    # weight + x loads first (gpsimd is only safe to use
                    # BEFORE the first collective trigger in program order)
                    wqk_t = whp.tile([128, DT, QK], bf16, tag="wqk")
                    nc.sync.dma_start(out=wqk_t, in_=wqk_r)
                    xg = xgp.tile([128, DT, IC, 512], bf16, tag="xg")
                    for jg in range(IC):
                        for dh in range(2):
                            eng = nc.sync if (jg + dh) % 2 else nc.gpsimd
                            eng.dma_start(
                                out=xg[:, dh * 4:(dh + 1) * 4, jg, :],
                                in_=xT_r[:, dh * 4:(dh + 1) * 4, jg, :])
                    wh_t = whp.tile([128, DT, 2 * H], bf16, tag="wh")
                    for dt in range(DT):
                        eng = nc.gpsimd if dt % 2 else nc.scalar
                        eng.dma_start(out=wh_t[:, dt, :], in_=wh_r[:, dt, :])

                    # ---- qk -> k (own rows) + q (own rows) ----
                    for jg in range(IC):
                        for ct in range(2):
                            cw = 128 if ct == 0 else QK - 128
                            pq = ps_qk.tile([128, 512], f32)
                            for dt in range(DT):
                                nc.tensor.matmul(
                                    pq[:cw],
                                    wqk_t[:, dt, ct * 128:ct * 128 + cw],
                                    xg[:, dt, jg, :],
                                    start=(dt == 0),
                                    stop=(dt == DT - 1),
                                )
                            sil = st1.tile([128, 512], f32, tag="sil")
                            nc.scalar.activation(
                                sil[:cw], pq[:cw], ACT.Silu,
                                bias=bqk_t[:cw, ct:ct + 1],
                            )
                            kt = st1.tile([128, 512], f32r, tag="kt")
                            nc.vector.tensor_scalar(
                                out=kt[:cw], in0=sil[:cw],
                                scalar1=gk_t[:cw, ct:ct + 1],
                                scalar2=bk_t[:cw, ct:ct + 1],
                                op0=ALU.mult, op1=ALU.add,
                            )
                            nc.sync.dma_start(
                                out=kT_own[ct, 0:cw, jg * 512:(jg + 1) * 512],
                                in_=kt[:cw],
                            )
                            nc.vector.tensor_scalar(
                                out=qT_s[:cw, ct, jg * 512:(jg + 1) * 512],
                                in0=sil[:cw],
                                scalar1=gq_t[:cw, ct:ct + 1],
                                scalar2=bq_t[:cw, ct:ct + 1],
                                op0=ALU.mult, op1=ALU.add,
                            )
                    # k slice done -> gather all k (fast, hides under v/gate)
                    nc.gpsimd.collective_compute(
                        "AllGather", ALU.bypass, replica_groups=RG,
                        ins=[kT_own.opt()], outs=[kT_ag.opt()],
                    )

                    # ---- v (own rows, row-major for the AG) ----
                    for jg in range(IC):
                        for jt in range(4):
                            for hc in range(4):
                                pv = ps_v.tile([128, 512], f32)
                                for dt in range(DT):
                                    nc.tensor.matmul(
                                        pv,
                                        xg[:, dt, jg, jt * 128:(jt + 1) * 128],
                                        wh_t[:, dt, hc * 512:(hc + 1) * 512],
                                        start=(dt == 0),
                                        stop=(dt == DT - 1),
                                    )
                                vt = st1.tile([128, 512], bf16, tag="vt")
                                if vbias:
                                    tmp = st1.tile([128, 512], f32, tag="vtmp")
                                    nc.vector.tensor_add(
                                        tmp, pv, bv_t[:, hc * 512:(hc + 1) * 512])
                                    nc.scalar.activation(vt, tmp, ACT.Silu)
                                else:
                                    nc.scalar.activation(vt, pv, ACT.Silu)
                                veng = nc.sync if (jt + hc) % 2 else nc.scalar
                                veng.dma_start(
                                    out=v_own[(jg * 4 + jt) * 128:
                                              (jg * 4 + jt + 1) * 128,
                                              hc * 512:(hc + 1) * 512],
                                    in_=vt,
                                )
                    # v slice done -> gather all v (long pole; hides under
                    # the gate matmuls + attn generation)
                    nc.gpsimd.collective_compute(
                        "AllGather", ALU.bypass, replica_groups=RG,
                        ins=[v_own.opt()], outs=[v_ag.opt()],
                    )

                    # ---- gateT (own rows) ----
                    for jg in range(IC):
                        for ht in range(HT):
                            pg = ps_g.tile([128, 512], f32)
                            for dt in range(DT):
                                nc.tensor.matmul(
                                    pg,
                                    wh_t[:, dt, H + ht * 128:H + (ht + 1) * 128],
                                    xg[:, dt, jg, :],
                                    start=(dt == 0),
                                    stop=(dt == DT - 1),
                                )
                            gt = st1.tile([128, 512], bf16, tag="gt")
                            nc.scalar.activation(
                                gt, pg, ACT.Silu, bias=bg_t[:, ht:ht + 1])
                            geng = nc.sync if (jg + ht) % 2 else nc.scalar
                            geng.dma_start(
                                out=gT_d[ht, :, jg * 512:(jg + 1) * 512],
                                in_=gt,
                            )

                # ============ phase 2: attention, all PSUM via one tag ============
                # "acc" slots are [128, 2048] fp32 = 4 banks, bufs=2 -> 8 banks.
                with (
                    tc.tile_pool(name="p2sb", bufs=1) as p2sb,
                    tc.tile_pool(name="kqp", bufs=2) as kqp,
                    tc.tile_pool(name="vst", bufs=4) as vst,
                    tc.tile_pool(name="gst", bufs=2) as gst,
                    tc.tile_pool(name="wop", bufs=2) as wop,
                    tc.tile_pool(name="xop", bufs=1) as xop,
                    tc.tile_pool(name="ost", bufs=2) as osp,
                    tc.tile_pool(name="p2ps", bufs=2, space="PSUM") as p2ps,
                ):
                    # attn for BOTH i-chunks: [j-part, jt, i] over all own rows
                    attn = p2sb.tile([128, JT, R], bf16, tag="attn",
                                     name="attn")
                    gated = p2sb.tile([128, HT, R], bf16, tag="gated",
                                      name="gated")

                    # ---- A: attn[j, :] = relu(k.T q)^2, one jg batch at a time ----
                    if do_pA:
                        for ic in range(IC):
                            q_sb = qT_s[:, :, ic * 512:(ic + 1) * 512]
                            for jg in range(JT // 4):
                                kt_sb = kqp.tile([128, 2, 512], f32r,
                                                 tag="kt_sb", name="kt_sb")
                                keng = nc.sync if jg % 2 else nc.scalar
                                keng.dma_start(
                                    out=kt_sb,
                                    in_=kT_ag[jg // 2, :, :,
                                              (jg % 2) * 512:(jg % 2 + 1) * 512]
                                    .rearrange("ct c j -> c ct j"),
                                )
                                pss = p2ps.tile([128, 4, 512], f32, tag="acc",
                                                name="pss")
                                for j4 in range(4):
                                    nc.tensor.matmul(
                                        pss[:, j4, :],
                                        kt_sb[:, 0, j4 * 128:(j4 + 1) * 128],
                                        q_sb[:, 0, :], start=True, stop=False)
                                    nc.tensor.matmul(
                                        pss[:, j4, :],
                                        kt_sb[0:QK - 128, 1,
                                              j4 * 128:(j4 + 1) * 128],
                                        q_sb[0:QK - 128, 1, :],
                                        start=False, stop=True)
                                rel = kqp.tile([128, 4, 512], bf16,
                                               tag="rel", name="rel")
                                nc.scalar.activation(rel, pss, ACT.Relu)
                                nc.vector.tensor_mul(
                                    attn[:, 4 * jg:4 * jg + 4,
                                         ic * 512:(ic + 1) * 512],
                                    rel, rel)

                        if not (do_pB and do_pC):
                            pa = kqp.tile([128, 512], f32, tag="pa",
                                          bufs=1, name="pa")
                            nc.vector.tensor_copy(pa, attn[:, 0, 0:512])
                            nc.sync.dma_start(
                                out=out.rearrange("(a p) m -> p a m", p=128)
                                [:, 1, 0:512], in_=pa)

                    # ---- B: out1T[h, :] = v-lhsT @ attn; * gateT ----
                    # h-groups of 2 tiles x both i-chunks per 4-bank slot;
                    # each v tile is read exactly once.
                    if do_pB:
                        for hg in range(HT // 2):
                            po = p2ps.tile([128, 2, IC, 512], f32, tag="acc",
                                           name="po")
                            for jt in range(JT):
                                vt = vst.tile([128, 256], bf16, tag="vt",
                                              name="vt")
                                veng = nc.sync if jt % 2 else nc.scalar
                                veng.dma_start(
                                    out=vt,
                                    in_=v_ag[jt * 128:(jt + 1) * 128,
                                             hg * 256:(hg + 1) * 256],
                                )
                                for hh in range(2):
                                    for ic2 in range(IC):
                                        nc.tensor.matmul(
                                            po[:, hh, ic2, :],
                                            vt[:, hh * 128:(hh + 1) * 128],
                                            attn[:, jt,
                                                 ic2 * 512:(ic2 + 1) * 512],
                                            start=(jt == 0),
                                            stop=(jt == JT - 1),
                                        )
                            for hh in range(2):
                                ht = hg * 2 + hh
                                gt = gst.tile([128, R], bf16, tag="gt",
                                              name="gt")
                                geng = nc.sync if ht % 2 else nc.scalar
                                geng.dma_start(out=gt, in_=gT_d[ht, :, :])
                                nc.vector.tensor_mul(
                                    gated[:, ht, :],
                                    po[:, hh, :, :].rearrange("p a b -> p (a b)"),
                                    gt)

                        if not do_pC:
                            pb = gst.tile([128, 512], f32, tag="pb",
                                          bufs=1, name="pb")
                            nc.vector.tensor_copy(pb, gated[:, 0, 0:512])
                            nc.sync.dma_start(
                                out=out.rearrange("(a p) m -> p a m", p=128)
                                [:, 4, 0:512], in_=pb)

                    # ---- C: out2 = gatedT.T @ W_out; out = out2 * x ----
                    if do_pC:
                        for mc in range(2):
                            pos0 = p2ps.tile([128, 4, 512], f32, tag="acc",
                                             name="pos0")
                            pos1 = p2ps.tile([128, 4, 512], f32, tag="acc",
                                             name="pos1")
                            poss = (pos0, pos1)
                            for hq in range(4):
                                wo = wop.tile([128, 4, 512], bf16, tag="wo",
                                              name="wo")
                                for dh in range(2):
                                    eng = nc.sync if (hq + dh) % 2 else nc.scalar
                                    eng.dma_start(
                                        out=wo[:, dh * 2:(dh + 1) * 2, :],
                                        in_=wo_r[:, hq * 4 + dh * 2:
                                                 hq * 4 + (dh + 1) * 2,
                                                 mc * 512:(mc + 1) * 512],
                                    )
                                for h4 in range(4):
                                    for it in range(8):
                                        nc.tensor.matmul(
                                            poss[it // 4][:, it % 4, :],
                                            gated[:, hq * 4 + h4,
                                                  it * 128:(it + 1) * 128],
                                            wo[:, h4, :],
                                            start=(hq == 0 and h4 == 0),
                                            stop=(hq == 3 and h4 == 3),
                                        )
                            for it in range(8):
                                xo = xop.tile([128, 1024], f32, tag="xo",
                                              name="xo")
                                xeng = nc.sync if it % 2 else nc.scalar
                                xeng.dma_start(out=xo, in_=xo_r[:, it, :])
                                ot = osp.tile([128, 512], f32, tag="ot",
                                              name="ot")
                                if obias:
                                    nc.vector.tensor_add(
                                        ot, poss[it // 4][:, it % 4, :],
                                        bo_t[:, mc * 512:(mc + 1) * 512])
                                    nc.vector.tensor_mul(
                                        ot, ot,
                                        xo[:, mc * 512:(mc + 1) * 512])
                                else:
                                    nc.vector.tensor_mul(
                                        ot, poss[it // 4][:, it % 4, :],
                                        xo[:, mc * 512:(mc + 1) * 512])
                                oeng = nc.sync if it % 2 else nc.scalar
                                oeng.dma_start(
                                    out=out.rearrange("(it p) m -> p it m",
                                                      p=128)
                                    [:, it, mc * 512:(mc + 1) * 512],
                                    in_=ot,
                                )

            # anchor outputs for phase-subset timing builds (prevents DCE)
            if not (do_pA and do_pB and do_pC):
                tc.strict_bb_all_engine_barrier()
                with tc.tile_pool(name="probe", bufs=1) as prp:
                    if do_p1:
                        pt = prp.tile([128, 512], f32)
                        nc.sync.dma_start(
                            out=pt, in_=v_ags[-1][0:128, 0:256].bitcast(f32))
                        nc.sync.dma_start(
                            out=out.rearrange("(a p) m -> p a m", p=128)
                            [:, 0, 0:512], in_=pt)

    nc.compile()
    return nc


# ---------------------------------------------------------------- runner ----

import time as _time

import jax
import jax.numpy as jnp
from jax.sharding import Mesh, NamedSharding, PartitionSpec
from jax.experimental.shard_map import shard_map

from concourse.bass2jax import _bass_exec_p, install_neuronx_cc_hook, partition_id_tensor


class SpmdRunner:
    def __init__(self, nc, n_cores=8):
        install_neuronx_cc_hook()
        self.nc = nc
        self.n_cores = n_cores
        partition_name = nc.partition_id_tensor.name if nc.partition_id_tensor else None
        in_names, out_names, out_avals, zero_outs = [], [], [], []
        for alloc in nc.m.functions[0].allocations:
            if not isinstance(alloc, mybir.MemoryLocationSet):
                continue
            name = alloc.memorylocations[0].name
            if alloc.kind == "ExternalInput":
                if name != partition_name:
                    in_names.append(name)
            elif alloc.kind == "ExternalOutput":
                shape = tuple(alloc.tensor_shape)
                dtype = mybir.dt.np(alloc.dtype)
                out_names.append(name)
                out_avals.append(jax.core.ShapedArray(shape, dtype))
                zero_outs.append(np.zeros(shape, dtype))
        self.in_names, self.out_names = in_names, out_names
        self.out_avals, self.zero_outs = out_avals, zero_outs
        n_params = len(in_names)
        all_names = in_names + out_names
        if partition_name is not None:
            all_names = all_names + [partition_name]

        def _body(*args):
            operands = list(args)
            if partition_name is not None:
                operands.append(partition_id_tensor())
            outs = _bass_exec_p.bind(
                *operands,
                out_avals=tuple(out_avals),
                in_names=tuple(all_names),
                out_names=tuple(out_names),
                lowering_input_output_aliases=(),
                sim_require_finite=True,
                sim_require_nnan=True,
                nc=nc,
            )
            return tuple(outs)

        devices = jax.devices()[:n_cores]
        self.mesh = Mesh(np.asarray(devices), ("core",))
        in_specs = (PartitionSpec("core"),) * (n_params + len(out_names))
        out_specs = (PartitionSpec("core"),) * len(out_names)
        self.sharded = jax.jit(
            shard_map(_body, mesh=self.mesh, in_specs=in_specs,
                      out_specs=out_specs, check_rep=False),
            keep_unused=True,
        )

    def stage_inputs(self, in_maps):
        n = self.n_cores
        concat = [
            np.concatenate([np.asarray(in_maps[c][name]) for c in range(n)], axis=0)
            for name in self.in_names
        ]
        concat += [np.zeros((n * z.shape[0], *z.shape[1:]), z.dtype)
                   for z in self.zero_outs]
        sharding = NamedSharding(self.mesh, PartitionSpec("core"))
        return [jax.device_put(a, sharding) for a in concat]

    def run(self, staged):
        outs = self.sharded(*staged)
        jax.block_until_ready(outs)
        return outs

    def run_numpy(self, staged):
        outs = self.run(staged)
        n = self.n_cores
        return [
            {name: np.asarray(outs[i]).reshape(n, *self.out_avals[i].shape)[c]
             for i, name in enumerate(self.out_names)}
            for c in range(n)
        ]


# ------------------------------------------------------------- host side ----

_CACHE = {}


def _get_runner(reps, vbias, obias):
    key = (reps, vbias, obias)
    if key not in _CACHE:
        nc = _build_nc(reps=reps, vbias=vbias, obias=obias)
        _CACHE[key] = SpmdRunner(nc, NC)
    return _CACHE[key]


def _pad2(v):
    o = np.zeros((2, 128), np.float32)
    o[0] = v[:128]
    o[1, :QK - 128] = v[128:QK]
    return o


def make_in_maps(x, W_hidden, b_hidden, W_qk, b_qk, gamma, beta, W_out, b_out):
    bf16_np = mybir.dt.np(bf16)
    x = np.ascontiguousarray(np.asarray(x, np.float32))
    scale = 1.0 / np.sqrt(np.float32(D))
    gq = _pad2(np.asarray(gamma[0], np.float32) * scale)
    bq = _pad2(np.asarray(beta[0], np.float32) * scale)
    gk = _pad2(np.asarray(gamma[1], np.float32))
    bk = _pad2(np.asarray(beta[1], np.float32))
    bqk = _pad2(np.asarray(b_qk, np.float32))
    bg = np.ascontiguousarray(
        np.asarray(b_hidden[H:], np.float32).reshape(HT, 128))
    W_hidden = np.ascontiguousarray(
        np.asarray(W_hidden, np.float32).astype(bf16_np))
    W_qk = np.ascontiguousarray(
        np.asarray(W_qk, np.float32).astype(bf16_np))
    W_out = np.ascontiguousarray(
        np.asarray(W_out, np.float32).astype(bf16_np))
    bv = np.asarray(b_hidden[:H], np.float32)
    bo = np.asarray(b_out, np.float32)
    vbias = bool(np.any(bv))
    obias = bool(np.any(bo))

    xT = np.ascontiguousarray(x.T)
    in_maps = []
    for c in range(NC):
        m = {
            "xT_own": np.ascontiguousarray(xT[:, c * R:(c + 1) * R]).astype(bf16_np),
            "x_own": x[c * R:(c + 1) * R],
            "w_h": W_hidden,
            "w_qk": W_qk,
            "w_out": W_out,
            "gq": gq, "bq": bq, "gk": gk, "bk": bk, "bqk": bqk, "bg": bg,
        }
        if vbias:
            m["bv"] = bv
        if obias:
            m["bo"] = bo
        in_maps.append(m)
    return in_maps, vbias, obias


def kernel(x, W_hidden, b_hidden, W_qk, b_qk, gamma, beta, W_out, b_out):
    in_maps, vbias, obias = make_in_maps(
        x, W_hidden, b_hidden, W_qk, b_qk, gamma, beta, W_out, b_out)
    runner = _get_runner(1, vbias, obias)
    staged = runner.stage_inputs(in_maps)
    results = runner.run_numpy(staged)
    return np.concatenate([results[c]["out"] for c in range(NC)], axis=0)


# revision 12
# speedup vs baseline: 3.8838x; 2.0458x over previous
"""GAU (Gated Attention Unit) Trainium2 Bass kernel, 8-core sequence-parallel.

Reference computation (all fp32):
    hid  = silu(x @ W_hidden + b_hidden);  v, gate = split(hid, 2)
    qk   = silu(x @ W_qk + b_qk)
    q    = qk * gamma[0] + beta[0];  k = qk * gamma[1] + beta[1]
    attn = relu((q @ k.T) / sqrt(dim))^2
    out  = ((attn @ v) * gate) @ W_out + b_out
    return out * x

Sharding (v4): rows (N=8192) split across 8 cores, 1024 rows each. Each core
computes k / v / q / gate for its OWN rows only, then AllGathers k and v
across the 8 cores. The collectives run on TOPSP+SDMA (separate silicon), so
they overlap with PE compute; gpsimd carries ONLY the collective triggers so
no compute-pool barrier transitively waits on an AG completion.

Phase 2 computes attn for BOTH 512-row i-chunks (attn covers all 1024 own
rows), then one attn@v pass reads each v tile once, then the W_out
contraction. All PSUM goes through a single "acc" tag of 4-bank slots,
double-buffered (8 banks), so group boundaries overlap.

Precision: phase-1 matmuls bf16 (x, W_hidden, W_qk staged bf16); k/q f32r.
v and attn are fp8e4 (E4M3), which (a) halves the AG(v) payload and (b)
enables DoubleRow perf mode on the dominant attn@v contraction (256-deep,
2x PE rate). attn is scaled by 4 (q by 2) to sit in E4M3's sweet spot and
W_out is staged as W_out/4 in bf16 to compensate. Measured end-to-end max
rel err ~1e-2 against the fp32 reference (gate: 2e-2).

reps are pipelined (no global inter-rep barrier); the rep-invariant x tiles
and W_qk are loaded once up front so later reps start with zero lead-in.
"""

import numpy as np

import concourse.bass as bass
import concourse.mybir as mybir
import concourse.tile as tile
from concourse import bacc

N = 8192          # total rows
D = 1024          # model dim
QK = 200          # qk dim
H = 2048          # hidden (v/gate) dim
NC = 8            # cores
R = N // NC       # rows per core
DT = D // 128     # d-tiles
HT = H // 128     # h-tiles
IC = R // 512     # i-chunks per core (own j-groups of 512)
JT = N // 128     # total j-tiles

f32 = mybir.dt.float32
f32r = mybir.dt.float32r
bf16 = mybir.dt.bfloat16
fp8 = mybir.dt.float8e4
ACT = mybir.ActivationFunctionType
ALU = mybir.AluOpType
DR = mybir.MatmulPerfMode.DoubleRow
RG = [list(range(NC))]


def _build_nc(reps=1, vbias=False, obias=False,
              do_p1=True, do_pA=True, do_pB=True, do_pC=True):
    nc = bacc.Bacc("TRN2", target_bir_lowering=False, debug=False,
                   num_devices=NC)

    xT_own = nc.dram_tensor("xT_own", [D, R], bf16, kind="ExternalInput").ap()
    x_own = nc.dram_tensor("x_own", [R, D], f32, kind="ExternalInput").ap()
    w_h = nc.dram_tensor("w_h", [D, 2 * H], bf16, kind="ExternalInput").ap()
    w_qk = nc.dram_tensor("w_qk", [D, QK], bf16, kind="ExternalInput").ap()
    w_out = nc.dram_tensor("w_out", [H, D], bf16, kind="ExternalInput").ap()
    # per-c scalars, padded 200 -> [2, 128]
    gq = nc.dram_tensor("gq", [2, 128], f32, kind="ExternalInput").ap()
    bq = nc.dram_tensor("bq", [2, 128], f32, kind="ExternalInput").ap()
    gk = nc.dram_tensor("gk", [2, 128], f32, kind="ExternalInput").ap()
    bk = nc.dram_tensor("bk", [2, 128], f32, kind="ExternalInput").ap()
    bqk = nc.dram_tensor("bqk", [2, 128], f32, kind="ExternalInput").ap()
    bg = nc.dram_tensor("bg", [HT, 128], f32, kind="ExternalInput").ap()
    if vbias:
        bv = nc.dram_tensor("bv", [H], f32, kind="ExternalInput").ap()
    if obias:
        bo = nc.dram_tensor("bo", [D], f32, kind="ExternalInput").ap()
    out = nc.dram_tensor("out", [R, D], f32, kind="ExternalOutput").ap()

    with tile.TileContext(nc) as tc:
        with (
            tc.tile_pool(name="pers", bufs=1) as pers,
            tc.tile_pool(name="dram", bufs=1, space="DRAM") as dpool,
        ):
            # persistent small tiles
            gq_t = pers.tile([128, 2], f32)
            bq_t = pers.tile([128, 2], f32)
            gk_t = pers.tile([128, 2], f32)
            bk_t = pers.tile([128, 2], f32)
            bqk_t = pers.tile([128, 2], f32)
            bg_t = pers.tile([128, HT], f32)
            nc.sync.dma_start(out=gq_t, in_=gq.rearrange("ct c -> c ct"))
            nc.sync.dma_start(out=bq_t, in_=bq.rearrange("ct c -> c ct"))
            nc.sync.dma_start(out=gk_t, in_=gk.rearrange("ct c -> c ct"))
            nc.sync.dma_start(out=bk_t, in_=bk.rearrange("ct c -> c ct"))
            nc.sync.dma_start(out=bqk_t, in_=bqk.rearrange("ct c -> c ct"))
            nc.sync.dma_start(out=bg_t, in_=bg.rearrange("ht c -> c ht"))
            if vbias:
                bv_t = pers.tile([128, H], f32)
                nc.sync.dma_start(
                    out=bv_t,
                    in_=bass.AP(tensor=bv.tensor, offset=bv.offset,
                                ap=[[0, 128]] + list(bv.ap)),
                )
            if obias:
                bo_t = pers.tile([128, D], f32)
                nc.sync.dma_start(
                    out=bo_t,
                    in_=bass.AP(tensor=bo.tensor, offset=bo.offset,
                                ap=[[0, 128]] + list(bo.ap)),
                )

            # DRAM scratch: own-row slices (AG inputs, Local) and the
            # gathered full tensors (AG outputs, Shared; single-writer each,
            # so one pair per timing rep)
            kT_own = dpool.tile([2, 128, R], f32r, tag="kT_own")
            v_own = dpool.tile([R, H], fp8, tag="v_own")
            kT_ags = [dpool.tile([NC, 2, 128, R], f32r, tag=f"kT_ag{r}",
                                 name=f"kT_ag{r}", addr_space="Shared")
                      for r in range(reps)]
            v_ags = [dpool.tile([N, H], fp8, tag=f"v_ag{r}",
                                name=f"v_ag{r}", addr_space="Shared")
                     for r in range(reps)]
            gT_d = dpool.tile([HT, 128, R], bf16, tag="gT_d")
            # qT lives in SBUF for the whole kernel (1 MB)
            qT_s = pers.tile([128, 2, R], f32r, tag="qT_s", name="qT_s")

            xT_r = xT_own.rearrange("(dt p) (jg j) -> p dt jg j", p=128, j=512)
            wh_r = w_h.rearrange("(dt p) h -> p dt h", p=128)
            wqk_r = w_qk.rearrange("(dt p) c -> p dt c", p=128)
            wo_r = w_out.rearrange("(ht p) m -> p ht m", p=128)
            xo_r = x_own.rearrange("(it p) m -> p it m", p=128)

            # rep-invariant SBUF: x tiles + W_qk, loaded once
            wqk_t = pers.tile([128, DT, QK], bf16, tag="wqk", name="wqk_t")
            nc.sync.dma_start(out=wqk_t, in_=wqk_r)
            xg = pers.tile([128, DT, IC, 512], bf16, tag="xg", name="xg")
            for jg in range(IC):
                for dh in range(2):
                    eng = nc.sync if (jg + dh) % 2 else nc.scalar
                    eng.dma_start(
                        out=xg[:, dh * 4:(dh + 1) * 4, jg, :],
                        in_=xT_r[:, dh * 4:(dh + 1) * 4, jg, :])

            for rep in range(reps):
                kT_ag = kT_ags[rep]
                v_ag = v_ags[rep]
                # ============ phase 1: own-row k, q, v, gate + AGs ============
                if not do_p1:
                    pass
                else:
                 with (
                    tc.tile_pool(name="whp", bufs=1) as whp,
                    tc.tile_pool(name="st1", bufs=(2 if vbias else 3)) as st1,
                    tc.tile_pool(name="ps_qk", bufs=2, space="PSUM") as ps_qk,
                    tc.tile_pool(name="ps_v", bufs=2, space="PSUM") as ps_v,
                    tc.tile_pool(name="ps_g", bufs=2, space="PSUM") as ps_g,
                ):
                    wh_t = whp.tile([128, DT, 2 * H], bf16, tag="wh")
                    for dt in range(DT):
                        eng = nc.sync if dt % 2 else nc.scalar
                        eng.dma_start(out=wh_t[:, dt, :], in_=wh_r[:, dt, :])

                    # ---- qk -> k (own rows) + q (own rows) ----
                    for jg in range(IC):
                        for ct in range(2):
                            cw = 128 if ct == 0 else QK - 128
                            pq = ps_qk.tile([128, 512], f32)
                            for dt in range(DT):
                                nc.tensor.matmul(
                                    pq[:cw],
                                    wqk_t[:, dt, ct * 128:ct * 128 + cw],
                                    xg[:, dt, jg, :],
                                    start=(dt == 0),
                                    stop=(dt == DT - 1),
                                )
                            sil = st1.tile([128, 512], f32, tag="sil")
                            nc.scalar.activation(
                                sil[:cw], pq[:cw], ACT.Silu,
                                bias=bqk_t[:cw, ct:ct + 1],
                            )
                            kt = st1.tile([128, 512], f32r, tag="kt")
                            nc.vector.tensor_scalar(
                                out=kt[:cw], in0=sil[:cw],
                                scalar1=gk_t[:cw, ct:ct + 1],
                                scalar2=bk_t[:cw, ct:ct + 1],
                                op0=ALU.mult, op1=ALU.add,
                            )
                            nc.sync.dma_start(
                                out=kT_own[ct, 0:cw, jg * 512:(jg + 1) * 512],
                                in_=kt[:cw],
                            )
                            nc.vector.tensor_scalar(
                                out=qT_s[:cw, ct, jg * 512:(jg + 1) * 512],
                                in0=sil[:cw],
                                scalar1=gq_t[:cw, ct:ct + 1],
                                scalar2=bq_t[:cw, ct:ct + 1],
                                op0=ALU.mult, op1=ALU.add,
                            )
                    # k slice done -> gather all k (fast, hides under v/gate)
                    nc.gpsimd.collective_compute(
                        "AllGather", ALU.bypass, replica_groups=RG,
                        ins=[kT_own.opt()], outs=[kT_ag.opt()],
                    )

                    # ---- v (own rows, row-major fp8 for the AG) ----
                    for jg in range(IC):
                        for jt in range(4):
                            for hc in range(4):
                                pv = ps_v.tile([128, 512], f32)
                                for dt in range(DT):
                                    nc.tensor.matmul(
                                        pv,
                                        xg[:, dt, jg, jt * 128:(jt + 1) * 128],
                                        wh_t[:, dt, hc * 512:(hc + 1) * 512],
                                        start=(dt == 0),
                                        stop=(dt == DT - 1),
                                    )
                                vt = st1.tile([128, 512], fp8, tag="vt")
                                if vbias:
                                    tmp = st1.tile([128, 512], f32, tag="vtmp")
                                    nc.vector.tensor_add(
                                        tmp, pv, bv_t[:, hc * 512:(hc + 1) * 512])
                                    nc.scalar.activation(vt, tmp, ACT.Silu)
                                else:
                                    nc.scalar.activation(vt, pv, ACT.Silu)
                                veng = nc.sync if (jt + hc) % 2 else nc.scalar
                                veng.dma_start(
                                    out=v_own[(jg * 4 + jt) * 128:
                                              (jg * 4 + jt + 1) * 128,
                                              hc * 512:(hc + 1) * 512],
                                    in_=vt,
                                )
                    # v slice done -> gather all v (hides under gate + attn gen)
                    nc.gpsimd.collective_compute(
                        "AllGather", ALU.bypass, replica_groups=RG,
                        ins=[v_own.opt()], outs=[v_ag.opt()],
                    )

                    # ---- gateT (own rows) ----
                    for jg in range(IC):
                        for ht in range(HT):
                            pg = ps_g.tile([128, 512], f32)
                            for dt in range(DT):
                                nc.tensor.matmul(
                                    pg,
                                    wh_t[:, dt, H + ht * 128:H + (ht + 1) * 128],
                                    xg[:, dt, jg, :],
                                    start=(dt == 0),
                                    stop=(dt == DT - 1),
                                )
                            gt = st1.tile([128, 512], bf16, tag="gt")
                            nc.scalar.activation(
                                gt, pg, ACT.Silu, bias=bg_t[:, ht:ht + 1])
                            geng = nc.sync if (jg + ht) % 2 else nc.scalar
                            geng.dma_start(
                                out=gT_d[ht, :, jg * 512:(jg + 1) * 512],
                                in_=gt,
                            )

                # ============ phase 2: attention, all PSUM via one tag ============
                # "acc" slots are [128, 2048] fp32 = 4 banks, bufs=2 -> 8 banks.
                with (
                    tc.tile_pool(name="p2sb", bufs=1) as p2sb,
                    tc.tile_pool(name="kqp", bufs=2) as kqp,
                    tc.tile_pool(name="vst", bufs=4) as vst,
                    tc.tile_pool(name="gst", bufs=2) as gst,
                    tc.tile_pool(name="wop", bufs=2) as wop,
                    tc.tile_pool(name="xop", bufs=2) as xop,
                    tc.tile_pool(name="ost", bufs=2) as osp,
                    tc.tile_pool(name="p2ps", bufs=2, space="PSUM") as p2ps,
                ):
                    # attn (x4-scaled, fp8) for BOTH i-chunks over all own rows
                    attn = p2sb.tile([128, JT, R], fp8, tag="attn",
                                     name="attn")
                    gated = p2sb.tile([128, HT, R], bf16, tag="gated",
                                      name="gated")

                    # ---- A: attn[j, :] = relu(2 k.T q)^2, per-jg batches ----
                    if do_pA:
                        for ic in range(IC):
                            q_sb = qT_s[:, :, ic * 512:(ic + 1) * 512]
                            for jg in range(JT // 4):
                                kt_sb = kqp.tile([128, 2, 512], f32r,
                                                 tag="kt_sb", name="kt_sb")
                                keng = nc.sync if jg % 2 else nc.scalar
                                keng.dma_start(
                                    out=kt_sb,
                                    in_=kT_ag[jg // 2, :, :,
                                              (jg % 2) * 512:(jg % 2 + 1) * 512]
                                    .rearrange("ct c j -> c ct j"),
                                )
                                pss = p2ps.tile([128, 4, 512], f32, tag="acc",
                                                name="pss")
                                for j4 in range(4):
                                    nc.tensor.matmul(
                                        pss[:, j4, :],
                                        kt_sb[:, 0, j4 * 128:(j4 + 1) * 128],
                                        q_sb[:, 0, :], start=True, stop=False)
                                    nc.tensor.matmul(
                                        pss[:, j4, :],
                                        kt_sb[0:QK - 128, 1,
                                              j4 * 128:(j4 + 1) * 128],
                                        q_sb[0:QK - 128, 1, :],
                                        start=False, stop=True)
                                rel = kqp.tile([128, 4, 512], bf16,
                                               tag="rel", name="rel")
                                nc.scalar.activation(rel, pss, ACT.Relu)
                                nc.vector.tensor_mul(
                                    attn[:, 4 * jg:4 * jg + 4,
                                         ic * 512:(ic + 1) * 512],
                                    rel, rel)

                        if not (do_pB and do_pC):
                            pa = kqp.tile([128, 512], f32, tag="pa",
                                          bufs=1, name="pa")
                            nc.vector.tensor_copy(pa, attn[:, 0, 0:512])
                            nc.sync.dma_start(
                                out=out.rearrange("(a p) m -> p a m", p=128)
                                [:, 1, 0:512], in_=pa)

                    # ---- B: out1T[h, :] = v-lhsT @ attn (fp8 DoubleRow,
                    # 256-deep contraction); then * gateT. Each v tile is
                    # read exactly once. ----
                    if do_pB:
                        for hg in range(HT // 2):
                            po = p2ps.tile([128, 2, IC, 512], f32, tag="acc",
                                           name="po")
                            for jt2 in range(JT // 2):
                                vt = vst.tile([128, 2, 256], fp8, tag="vt",
                                              name="vt")
                                veng = nc.sync if jt2 % 2 else nc.scalar
                                veng.dma_start(
                                    out=vt,
                                    in_=v_ag[jt2 * 256:(jt2 + 1) * 256,
                                             hg * 256:(hg + 1) * 256]
                                    .rearrange("(ko k) h -> k ko h", k=128),
                                )
                                for hh in range(2):
                                    for ic2 in range(IC):
                                        nc.tensor.matmul(
                                            po[:, hh, ic2, :],
                                            vt[:, :, hh * 128:(hh + 1) * 128],
                                            attn[:, 2 * jt2:2 * jt2 + 2,
                                                 ic2 * 512:(ic2 + 1) * 512],
                                            start=(jt2 == 0),
                                            stop=(jt2 == JT // 2 - 1),
                                            perf_mode=DR,
                                        )
                            for hh in range(2):
                                ht = hg * 2 + hh
                                gt = gst.tile([128, R], bf16, tag="gt",
                                              name="gt")
                                geng = nc.sync if ht % 2 else nc.scalar
                                geng.dma_start(out=gt, in_=gT_d[ht, :, :])
                                nc.vector.tensor_mul(
                                    gated[:, ht, :],
                                    po[:, hh, :, :].rearrange("p a b -> p (a b)"),
                                    gt)

                        if not do_pC:
                            pb = gst.tile([128, 512], f32, tag="pb",
                                          bufs=1, name="pb")
                            nc.vector.tensor_copy(pb, gated[:, 0, 0:512])
                            nc.sync.dma_start(
                                out=out.rearrange("(a p) m -> p a m", p=128)
                                [:, 4, 0:512], in_=pb)

                    # ---- C: out2 = gatedT.T @ (W_out/4); out = out2 * x ----
                    if do_pC:
                        for mc in range(2):
                            pos0 = p2ps.tile([128, 4, 512], f32, tag="acc",
                                             name="pos0")
                            pos1 = p2ps.tile([128, 4, 512], f32, tag="acc",
                                             name="pos1")
                            poss = (pos0, pos1)
                            for hq in range(4):
                                wo = wop.tile([128, 4, 512], bf16, tag="wo",
                                              name="wo")
                                for dh in range(2):
                                    eng = nc.sync if (hq + dh) % 2 else nc.scalar
                                    eng.dma_start(
                                        out=wo[:, dh * 2:(dh + 1) * 2, :],
                                        in_=wo_r[:, hq * 4 + dh * 2:
                                                 hq * 4 + (dh + 1) * 2,
                                                 mc * 512:(mc + 1) * 512],
                                    )
                                for h4 in range(4):
                                    for it in range(8):
                                        nc.tensor.matmul(
                                            poss[it // 4][:, it % 4, :],
                                            gated[:, hq * 4 + h4,
                                                  it * 128:(it + 1) * 128],
                                            wo[:, h4, :],
                                            start=(hq == 0 and h4 == 0),
                                            stop=(hq == 3 and h4 == 3),
                                        )
                            for it in range(8):
                                xo = xop.tile([128, 1024], f32, tag="xo",
                                              name="xo")
                                xeng = nc.sync if it % 2 else nc.scalar
                                xeng.dma_start(out=xo, in_=xo_r[:, it, :])
                                ot = osp.tile([128, 512], f32, tag="ot",
                                              name="ot")
                                if obias:
                                    nc.vector.tensor_add(
                                        ot, poss[it // 4][:, it % 4, :],
                                        bo_t[:, mc * 512:(mc + 1) * 512])
                                    nc.vector.tensor_mul(
                                        ot, ot,
                                        xo[:, mc * 512:(mc + 1) * 512])
                                else:
                                    nc.vector.tensor_mul(
                                        ot, poss[it // 4][:, it % 4, :],
                                        xo[:, mc * 512:(mc + 1) * 512])
                                oeng = nc.sync if it % 2 else nc.scalar
                                oeng.dma_start(
                                    out=out.rearrange("(it p) m -> p it m",
                                                      p=128)
                                    [:, it, mc * 512:(mc + 1) * 512],
                                    in_=ot,
                                )

            # anchor outputs for phase-subset timing builds (prevents DCE)
            if not (do_pA and do_pB and do_pC):
                tc.strict_bb_all_engine_barrier()
                with tc.tile_pool(name="probe", bufs=1) as prp:
                    if do_p1:
                        pt = prp.tile([128, 512], f32)
                        nc.sync.dma_start(
                            out=pt, in_=v_ags[-1][0:128, 0:2048].bitcast(f32))
                        nc.sync.dma_start(
                            out=out.rearrange("(a p) m -> p a m", p=128)
                            [:, 0, 0:512], in_=pt)

    nc.compile()
    return nc


# ---------------------------------------------------------------- runner ----

import time as _time

import jax
import jax.numpy as jnp
from jax.sharding import Mesh, NamedSharding, PartitionSpec
from jax.experimental.shard_map import shard_map

from concourse.bass2jax import _bass_exec_p, install_neuronx_cc_hook, partition_id_tensor


class SpmdRunner:
    def __init__(self, nc, n_cores=8):
        install_neuronx_cc_hook()
        self.nc = nc
        self.n_cores = n_cores
        partition_name = nc.partition_id_tensor.name if nc.partition_id_tensor else None
        in_names, out_names, out_avals, zero_outs = [], [], [], []
        for alloc in nc.m.functions[0].allocations:
            if not isinstance(alloc, mybir.MemoryLocationSet):
                continue
            name = alloc.memorylocations[0].name
            if alloc.kind == "ExternalInput":
                if name != partition_name:
                    in_names.append(name)
            elif alloc.kind == "ExternalOutput":
                shape = tuple(alloc.tensor_shape)
                dtype = mybir.dt.np(alloc.dtype)
                out_names.append(name)
                out_avals.append(jax.core.ShapedArray(shape, dtype))
                zero_outs.append(np.zeros(shape, dtype))
        self.in_names, self.out_names = in_names, out_names
        self.out_avals, self.zero_outs = out_avals, zero_outs
        n_params = len(in_names)
        all_names = in_names + out_names
        if partition_name is not None:
            all_names = all_names + [partition_name]

        def _body(*args):
            operands = list(args)
            if partition_name is not None:
                operands.append(partition_id_tensor())
            outs = _bass_exec_p.bind(
                *operands,
                out_avals=tuple(out_avals),
                in_names=tuple(all_names),
                out_names=tuple(out_names),
                lowering_input_output_aliases=(),
                sim_require_finite=True,
                sim_require_nnan=True,
                nc=nc,
            )
            return tuple(outs)

        devices = jax.devices()[:n_cores]
        self.mesh = Mesh(np.asarray(devices), ("core",))
        in_specs = (PartitionSpec("core"),) * (n_params + len(out_names))
        out_specs = (PartitionSpec("core"),) * len(out_names)
        self.sharded = jax.jit(
            shard_map(_body, mesh=self.mesh, in_specs=in_specs,
                      out_specs=out_specs, check_rep=False),
            keep_unused=True,
        )

    def stage_inputs(self, in_maps):
        n = self.n_cores
        concat = [
            np.concatenate([np.asarray(in_maps[c][name]) for c in range(n)], axis=0)
            for name in self.in_names
        ]
        concat += [np.zeros((n * z.shape[0], *z.shape[1:]), z.dtype)
                   for z in self.zero_outs]
        sharding = NamedSharding(self.mesh, PartitionSpec("core"))
        return [jax.device_put(a, sharding) for a in concat]

    def run(self, staged):
        outs = self.sharded(*staged)
        jax.block_until_ready(outs)
        return outs

    def run_numpy(self, staged):
        outs = self.run(staged)
        n = self.n_cores
        return [
            {name: np.asarray(outs[i]).reshape(n, *self.out_avals[i].shape)[c]
             for i, name in enumerate(self.out_names)}
            for c in range(n)
        ]


# ------------------------------------------------------------- host side ----

_CACHE = {}


def _get_runner(reps, vbias, obias):
    key = (reps, vbias, obias)
    if key not in _CACHE:
        nc = _build_nc(reps=reps, vbias=vbias, obias=obias)
        _CACHE[key] = SpmdRunner(nc, NC)
    return _CACHE[key]


def _pad2(v):
    o = np.zeros((2, 128), np.float32)
    o[0] = v[:128]
    o[1, :QK - 128] = v[128:QK]
    return o


# attn is computed as relu(2*sim)^2 = 4*relu(sim)^2 to center E4M3;
# compensated by staging W_out/4.
ATTN_SCALE = 2.0


def make_in_maps(x, W_hidden, b_hidden, W_qk, b_qk, gamma, beta, W_out, b_out):
    bf16_np = mybir.dt.np(bf16)
    x = np.ascontiguousarray(np.asarray(x, np.float32))
    scale = ATTN_SCALE / np.sqrt(np.float32(D))
    gq = _pad2(np.asarray(gamma[0], np.float32) * scale)
    bq = _pad2(np.asarray(beta[0], np.float32) * ATTN_SCALE)
    gk = _pad2(np.asarray(gamma[1], np.float32))
    bk = _pad2(np.asarray(beta[1], np.float32))
    bqk = _pad2(np.asarray(b_qk, np.float32))
    bg = np.ascontiguousarray(
        np.asarray(b_hidden[H:], np.float32).reshape(HT, 128))
    W_hidden = np.ascontiguousarray(
        np.asarray(W_hidden, np.float32).astype(bf16_np))
    W_qk = np.ascontiguousarray(
        np.asarray(W_qk, np.float32).astype(bf16_np))
    W_out = np.ascontiguousarray(
        (np.asarray(W_out, np.float32) / (ATTN_SCALE * ATTN_SCALE))
        .astype(bf16_np))
    bv = np.asarray(b_hidden[:H], np.float32)
    bo = np.asarray(b_out, np.float32)
    vbias = bool(np.any(bv))
    obias = bool(np.any(bo))

    xT = np.ascontiguousarray(x.T)
    in_maps = []
    for c in range(NC):
        m = {
            "xT_own": np.ascontiguousarray(
                xT[:, c * R:(c + 1) * R]).astype(bf16_np),
            "x_own": x[c * R:(c + 1) * R],
            "w_h": W_hidden,
            "w_qk": W_qk,
            "w_out": W_out,
            "gq": gq, "bq": bq, "gk": gk, "bk": bk, "bqk": bqk, "bg": bg,
        }
        if vbias:
            m["bv"] = bv
        if obias:
            m["bo"] = bo
        in_maps.append(m)
    return in_maps, vbias, obias


def kernel(x, W_hidden, b_hidden, W_qk, b_qk, gamma, beta, W_out, b_out):
    in_maps, vbias, obias = make_in_maps(
        x, W_hidden, b_hidden, W_qk, b_qk, gamma, beta, W_out, b_out)
    runner = _get_runner(1, vbias, obias)
    staged = runner.stage_inputs(in_maps)
    results = runner.run_numpy(staged)
    return np.concatenate([results[c]["out"] for c in range(NC)], axis=0)
